# revision 2
# baseline (speedup 1.0000x reference)
"""Multi-head attention (RoPE, softmax, out-proj) on 8 Trainium2 NeuronCores.

The tunnel between host and the axon-attached devices runs at ~40 MB/s, so
the wall time of run_bass_kernel_spmd is dominated by bytes crossing it, not
by device compute. This kernel is organized so every byte crosses exactly
once, in bf16:

  - tensor-parallel over all 8 cores: core c owns heads {2c, 2c+1}
    (column-parallel wq/wk/wv, row-parallel wo), and processes BOTH batches
    for those heads. Weight slices are disjoint across cores (no duplicate
    upload).
  - x (transposed, both batches stacked: [4096, S]) and the RoPE cos/sin
    rows ([256, S]) are sharded row-wise 8 ways, packed into one [544, S]
    bf16 tensor per core, and AllGathered on device (on-chip collective,
    ~70us) instead of being replicated over the tunnel.
  - each core's partial out-projection ([2S, D] f32) is ReduceScattered
    (add) across the 8 cores; each core converts its [2S/8, D] shard to
    bf16 and returns only that. The host concatenates the shards.

Compute structure per core is the proven head-group pipeline (matmuls in
bf16 at full PE rate with fp32 PSUM accumulation; RoPE as a partition-block
half-swap with host-permuted q/k feature rows and [+sin;-sin] sign folding;
softmax unnormalized in exp with the denominator reduced by an fp32r
ones-matmul and applied as a reciprocal multiply).
"""
import math
import sys

import numpy as np

for _p in ('/opt/trn_rl_repo', '/root/.axon_site/_ro/trn_rl_repo'):
    if _p not in sys.path:
        sys.path.insert(0, _p)

import ml_dtypes
import orjson

import concourse.bass as bass
import concourse.mybir as mybir
from concourse.tile import TileContext
from concourse.bass_utils import run_bass_kernel_spmd

F32 = mybir.dt.float32
R32 = mybir.dt.float32r
BF16 = mybir.dt.bfloat16
NP_BF16 = ml_dtypes.bfloat16

B = 2
S = 2048
D = 2048
HD = 128
N_HEADS = D // HD   # 16
N_CORES = 8
HPC = N_HEADS // N_CORES   # heads per core (2)
LF = HPC * HD              # local features per core (256)
XROWS = B * D // N_CORES   # x-shard rows per core (512)
CSROWS = 2 * HD // N_CORES  # cos/sin shard rows per core (32)
GROWS = XROWS + CSROWS     # packed gather-input rows (544)


# ---------------------------------------------------------------------------
# Wait-splitting post-pass: this toolchain's walrus supports at most ONE sync
# wait command per instruction (none at all on fp32/fp32r Matmult, which
# lowers to an LDW+MM pair). Tile emits multi-wait instructions; hoist the
# excess onto NoOps on the same engine immediately before the instruction.
# ---------------------------------------------------------------------------

def _keep_count(ins):
    if ins.get('opcode') == 'Matmult':
        dt = None
        for arg in ins.get('ins', []):
            dt = arg.get('dtype') or dt
        if dt in ('float32', 'float32r'):
            return 0
        return 1
    return 1


def _split_waits_json(data: bytes) -> bytes:
    d = orjson.loads(data)
    ctr = 0
    for fn in d.get('functions', []):
        for bb in fn.get('blocks', []):
            out = []
            for ins in bb.get('instructions', []):
                si = ins.get('sync_info')
                waits = (si or {}).get('on_wait') or []
                keep = _keep_count(ins)
                if len(waits) > keep:
                    hoist = waits[:len(waits) - keep]
                    keep_w = waits[len(waits) - keep:]
                    for w in hoist:
                        ctr += 1
                        nop = {
                            'name': f"{ins['name']}-ws{ctr}",
                            'opcode': 'NoOp',
                            'engine': ins.get('engine'),
                            'ins': [],
                            'outs': [],
                            'sync_info': {'on_wait': [w], 'on_update': []},
                        }
                        if 'debug' in ins:
                            nop['debug'] = ins['debug']
                        out.append(nop)
                    si['on_wait'] = keep_w
                out.append(ins)
            bb['instructions'] = out
    return orjson.dumps(d)


def _install_waitsplit():
    if getattr(bass.Bass, '_waitsplit_installed', False):
        return
    orig = bass.Bass.to_json_bytes

    def patched(self, *a, **k):
        return _split_waits_json(orig(self, *a, **k))

    bass.Bass.to_json_bytes = patched
    bass.Bass._waitsplit_installed = True


_install_waitsplit()


# ---------------------------------------------------------------------------
# Device program (SPMD, identical on all cores; per-core data differs)
# ---------------------------------------------------------------------------

def build_nc(s=S):
    d = D
    lf = LF
    hpc = HPC
    kd_n = d // 128          # contraction chunks for projections (16)
    nw = 512 if s >= 512 else s  # free-dim width per matmul
    nsq = s // nw            # wide column chunks
    ns = s // 128            # 128-row chunks
    nj = d // 512
    jw = 512
    ry = B * s // N_CORES    # output rows per core after reduce-scatter
    scale = 1.0 / math.sqrt(HD)
    rg = [list(range(N_CORES))]

    nc = bass.Bass()
    g_in = nc.dram_tensor("g_in", [GROWS, s], BF16, kind="ExternalInput")
    wqT = nc.dram_tensor("wqT", [d, lf], BF16, kind="ExternalInput")
    wkT = nc.dram_tensor("wkT", [d, lf], BF16, kind="ExternalInput")
    wvT = nc.dram_tensor("wvT", [d, lf], BF16, kind="ExternalInput")
    woT = nc.dram_tensor("woT", [lf, d], BF16, kind="ExternalInput")
    y = nc.dram_tensor("y", [ry, d], BF16, kind="ExternalOutput")

    def g_row(b, kd):
        # row in the gathered tensor of xT_all row b*D + kd*128
        a = b * d + kd * 128
        r, off = divmod(a, XROWS)
        return GROWS * r + off

    def cs_row(i):
        # row in the gathered tensor of csn_all row i (0:128 cos, 128:256 sin)
        r, off = divmod(i, CSROWS)
        return GROWS * r + XROWS + off

    with TileContext(nc) as tc:
        with tc.tile_pool(name="dram", bufs=1, space="DRAM") as dpool:
            g_bounce = dpool.tile([GROWS, s], BF16, name="g_bounce")
            gathered = dpool.tile([N_CORES * GROWS, s], BF16,
                                  addr_space="Shared", name="gathered")
            y_part = dpool.tile([B * s, d], F32, name="y_part")
            y_rs = dpool.tile([ry, d], F32, name="y_rs")

            nc.gpsimd.dma_start(out=g_bounce[:], in_=g_in[:])
            nc.gpsimd.collective_compute(
                "AllGather", mybir.AluOpType.bypass, replica_groups=rg,
                ins=[g_bounce.opt()], outs=[gathered.opt()])

            # Persistent SBUF residents: post-RoPE q/k and v for both batches
            # (4 virtual head-groups = 2 heads x 2 batches), and the fp32r
            # ones column used for the softmax denominator.
            with tc.tile_pool(name="persist", bufs=1) as per:
                qT_all = per.tile([128, B * hpc * s], BF16, name="qT_all")
                kT_all = per.tile([128, B * hpc * s], BF16, name="kT_all")
                v_all = per.tile([128, B * ns * lf], BF16, name="v_all")
                ones_f = per.tile([128, 128], F32, name="ones_f")
                nc.vector.memset(ones_f, 1.0)
                ones = per.tile([128, 128], R32, name="ones")
                nc.vector.tensor_copy(ones, ones_f)

                # ---------- Stage A: q/k/v projections + RoPE ----------
                with tc.tile_pool(name="wqk", bufs=1) as wpool, \
                     tc.tile_pool(name="xa", bufs=2) as xpool, \
                     tc.tile_pool(name="csp", bufs=1) as cspool, \
                     tc.tile_pool(name="rp", bufs=2) as rpool, \
                     tc.tile_pool(name="psA", bufs=4, space="PSUM") as pspool, \
                     tc.tile_pool(name="psAV", bufs=2, space="PSUM") as pvpool:
                    wq_sb = wpool.tile([128, kd_n * lf], BF16, name="wq_sb")
                    wk_sb = wpool.tile([128, kd_n * lf], BF16, name="wk_sb")
                    wv_sb = wpool.tile([128, kd_n * lf], BF16, name="wv_sb")
                    for kd in range(kd_n):
                        nc.sync.dma_start(out=wq_sb[:, kd * lf:(kd + 1) * lf],
                                          in_=wqT[kd * 128:(kd + 1) * 128, :])
                        nc.scalar.dma_start(out=wk_sb[:, kd * lf:(kd + 1) * lf],
                                            in_=wkT[kd * 128:(kd + 1) * 128, :])
                        nc.scalar.dma_start(out=wv_sb[:, kd * lf:(kd + 1) * lf],
                                            in_=wvT[kd * 128:(kd + 1) * 128, :])

                    # cos/sin: gathered bf16 rows -> SBUF -> f32. sn rows are
                    # [+sin; -sin] (host-prepared) so the half-swap cross
                    # terms land with the right signs.
                    cs_bf = cspool.tile([128, s], BF16, name="cs_bf")
                    sn_bf = cspool.tile([128, s], BF16, name="sn_bf")
                    for i in range(0, 128, CSROWS):
                        nc.sync.dma_start(
                            out=cs_bf[i:i + CSROWS, :],
                            in_=gathered[cs_row(i):cs_row(i) + CSROWS, :])
                        nc.sync.dma_start(
                            out=sn_bf[i:i + CSROWS, :],
                            in_=gathered[cs_row(128 + i):cs_row(128 + i) + CSROWS, :])
                    cs_sb = cspool.tile([128, s], F32, name="cs_sb")
                    sn_sb = cspool.tile([128, s], F32, name="sn_sb")
                    nc.vector.tensor_copy(cs_sb, cs_bf)
                    nc.vector.tensor_copy(sn_sb, sn_bf)

                    def load_x(b, sq):
                        t = xpool.tile([128, kd_n * nw], BF16, name="x_sb")
                        for kd in range(kd_n):
                            r = g_row(b, kd)
                            nc.sync.dma_start(
                                out=t[:, kd * nw:(kd + 1) * nw],
                                in_=gathered[r:r + 128, sq * nw:(sq + 1) * nw])
                        return t

                    def emit_v(b, sq, x_tile):
                        for ss in range(nw // 128):
                            psv = pvpool.tile([128, lf], F32, name="psv")
                            for kd in range(kd_n):
                                nc.tensor.matmul(
                                    psv,
                                    x_tile[:, kd * nw + ss * 128:
                                           kd * nw + (ss + 1) * 128],
                                    wv_sb[:, kd * lf:(kd + 1) * lf],
                                    start=(kd == 0), stop=(kd == kd_n - 1))
                            sk = sq * (nw // 128) + ss
                            nc.vector.tensor_copy(
                                v_all[:, (b * ns + sk) * lf:(b * ns + sk + 1) * lf],
                                psv)

                    x_prev = None
                    x_next = load_x(0, 0)
                    for bi in range(B):
                        for sq in range(nsq):
                            x_sb = x_next
                            if not (bi == B - 1 and sq == nsq - 1):
                                nb, nq = (bi, sq + 1) if sq + 1 < nsq else (bi + 1, 0)
                                x_next = load_x(nb, nq)
                            for wsb, dstT in ((wq_sb, qT_all), (wk_sb, kT_all)):
                                for h in range(hpc):
                                    g4 = bi * hpc + h
                                    ps = pspool.tile([128, nw], F32, name="ps_qk")
                                    for kd in range(kd_n):
                                        nc.tensor.matmul(
                                            ps,
                                            wsb[:, kd * lf + h * 128:
                                                kd * lf + (h + 1) * 128],
                                            x_sb[:, kd * nw:(kd + 1) * nw],
                                            start=(kd == 0), stop=(kd == kd_n - 1))
                                    tcc = rpool.tile([128, nw], F32, name="t_c")
                                    tss = rpool.tile([128, nw], F32, name="t_s")
                                    nc.vector.tensor_mul(
                                        tcc, ps, cs_sb[:, sq * nw:(sq + 1) * nw])
                                    nc.vector.tensor_mul(
                                        tss, ps, sn_sb[:, sq * nw:(sq + 1) * nw])
                                    tsw = rpool.tile([128, nw], F32, name="t_sw")
                                    nc.sync.dma_start(out=tsw[0:64, :],
                                                      in_=tss[64:128, :])
                                    nc.sync.dma_start(out=tsw[64:128, :],
                                                      in_=tss[0:64, :])
                                    nc.vector.tensor_add(
                                        dstT[:, g4 * s + sq * nw:
                                             g4 * s + sq * nw + nw], tcc, tsw)
                            if x_prev is not None:
                                pb, pq, pt = x_prev
                                emit_v(pb, pq, pt)
                            x_prev = (bi, sq, x_sb)
                    pb, pq, pt = x_prev
                    emit_v(pb, pq, pt)

                # ---------- Stage B+C: attention, then partial out-proj ----------
                with tc.tile_pool(name="exp", bufs=2) as expool, \
                     tc.tile_pool(name="nrm", bufs=2) as npool, \
                     tc.tile_pool(name="atp", bufs=2) as atpool, \
                     tc.tile_pool(name="wop", bufs=1) as wopool, \
                     tc.tile_pool(name="yop", bufs=3) as yopool, \
                     tc.tile_pool(name="psS", bufs=3, space="PSUM") as pssc, \
                     tc.tile_pool(name="psM", bufs=1, space="PSUM") as pssm, \
                     tc.tile_pool(name="psV", bufs=2, space="PSUM") as psov, \
                     tc.tile_pool(name="psC", bufs=2, space="PSUM") as psc:
                    wo_sb = wopool.tile([128, hpc * d], BF16, name="wo_sb")
                    for i in range(hpc):
                        nc.sync.dma_start(out=wo_sb[:, i * d:(i + 1) * d],
                                          in_=woT[i * 128:(i + 1) * 128, :])
                    nsub = nw // 128

                    def emit_c_part(bq, aT_tile, ssub):
                        # one query-row slice of the partial out-projection
                        bi, sq = bq
                        for jn in range(nj):
                            yps = psc.tile([128, jw], F32, name="yps")
                            for i in range(hpc):
                                nc.tensor.matmul(
                                    yps,
                                    aT_tile[:, i * nw + ssub * 128:
                                            i * nw + (ssub + 1) * 128],
                                    wo_sb[:, i * d + jn * jw:(i * d + (jn + 1) * jw)],
                                    start=(i == 0), stop=(i == hpc - 1))
                            yo = yopool.tile([128, jw], F32, name="yo")
                            nc.vector.tensor_copy(yo, yps)
                            r0 = bi * s + sq * nw + ssub * 128
                            nc.sync.dma_start(
                                out=y_part[r0:r0 + 128, jn * jw:(jn + 1) * jw],
                                in_=yo)

                    prev_c = None  # ((bi, sq), aT_tile) of the previous chunk
                    for bi in range(B):
                        for sq in range(nsq):
                            aT_sq = atpool.tile([128, hpc * nw], BF16, name="aT_sq")
                            for h in range(hpc):
                                g4 = bi * hpc + h
                                qT_sl = qT_all[:, g4 * s + sq * nw:
                                               g4 * s + (sq + 1) * nw]
                                ex_sb = expool.tile([128, ns * nw], BF16, name="ex_sb")
                                acc = npool.tile([128, nw], F32, name="acc")
                                pairs = []
                                for sk in range(ns):
                                    sps = pssc.tile([128, nw], F32, name="sps")
                                    nc.tensor.matmul(
                                        sps,
                                        kT_all[:, g4 * s + sk * 128:
                                               g4 * s + (sk + 1) * 128],
                                        qT_sl, start=True, stop=True)
                                    nc.scalar.activation(
                                        ex_sb[:, sk * nw:(sk + 1) * nw], sps,
                                        mybir.ActivationFunctionType.Exp,
                                        scale=scale)
                                    # pairwise level-0 exp sums on the
                                    # otherwise-idle GPSIMD engine
                                    if sk % 2 == 1:
                                        pr = npool.tile([128, nw], F32,
                                                        name=f"pr{sk // 2}")
                                        nc.gpsimd.tensor_add(
                                            pr, ex_sb[:, (sk - 1) * nw:sk * nw],
                                            ex_sb[:, sk * nw:(sk + 1) * nw])
                                        pairs.append(pr)
                                if ns == 1:
                                    nc.vector.tensor_copy(acc, ex_sb[:, 0:nw])
                                else:
                                    nc.vector.tensor_add(acc, pairs[0], pairs[1])
                                    for pr in pairs[2:]:
                                        nc.vector.tensor_add(acc, acc, pr)
                                ov = psov.tile([128, nw], F32, name="ov")
                                for sk in range(ns):
                                    nc.tensor.matmul(
                                        ov,
                                        v_all[:, (bi * ns + sk) * lf + h * 128:
                                              (bi * ns + sk) * lf + (h + 1) * 128],
                                        ex_sb[:, sk * nw:(sk + 1) * nw],
                                        start=(sk == 0), stop=(sk == ns - 1))
                                accr = npool.tile([128, nw], R32, name="accr")
                                nc.vector.tensor_copy(accr, acc)
                                # partition reduction + row broadcast of the
                                # softmax denominator
                                sm = pssm.tile([128, nw], F32, name="sm")
                                nc.tensor.matmul(sm, ones, accr, start=True,
                                                 stop=True)
                                rec = npool.tile([128, nw], F32, name="rec")
                                nc.vector.reciprocal(rec, sm)
                                nc.vector.tensor_mul(
                                    aT_sq[:, h * nw:(h + 1) * nw], ov, rec)
                                # interleave the PREVIOUS chunk's out-projection
                                # slices between heads
                                if prev_c is not None:
                                    pbq, pat = prev_c
                                    lo = h * nsub // hpc
                                    hi = (h + 1) * nsub // hpc
                                    for ssub in range(lo, hi):
                                        emit_c_part(pbq, pat, ssub)
                            prev_c = ((bi, sq), aT_sq)
                    pbq, pat = prev_c
                    for ssub in range(nsub):
                        emit_c_part(pbq, pat, ssub)

                # ---------- Stage D: reduce-scatter + bf16 convert ----------
                with tc.tile_pool(name="cvt", bufs=2) as cpool:
                    nc.gpsimd.collective_compute(
                        "ReduceScatter", mybir.AluOpType.add, replica_groups=rg,
                        ins=[y_part.opt()], outs=[y_rs.opt()])
                    for r0 in range(0, ry, 128):
                        yf = cpool.tile([128, d], F32, name="yf")
                        nc.sync.dma_start(out=yf, in_=y_rs[r0:r0 + 128, :])
                        yb = cpool.tile([128, d], BF16, name="yb")
                        nc.vector.tensor_copy(yb, yf)
                        nc.sync.dma_start(out=y[r0:r0 + 128, :], in_=yb)
    return nc


# ---------------------------------------------------------------------------
# Host-side prep + gather
# ---------------------------------------------------------------------------

_PERM_HEAD = np.concatenate([np.arange(0, HD, 2), np.arange(1, HD, 2)])


def _bf16(a):
    """Fast float32 -> bfloat16 with round-to-nearest-even (bit twiddling —
    ~4x faster than ml_dtypes astype on large arrays)."""
    a = np.ascontiguousarray(a, dtype=np.float32)
    u = a.view(np.uint32)
    r = ((u >> 16) & 1) + np.uint32(0x7FFF)
    return ((u + r) >> 16).astype(np.uint16).view(NP_BF16).reshape(a.shape)


def _prep_in_maps(x, wq, wk, wv, wo, pos_cos, pos_sin, s=S):
    d = D
    lf = LF
    # permute q/k feature rows within each head: even pairs first, then odd
    wq_p = wq.reshape(N_HEADS, HD, d)[:, _PERM_HEAD, :].reshape(d, d)
    wk_p = wk.reshape(N_HEADS, HD, d)[:, _PERM_HEAD, :].reshape(d, d)
    wqT_full = _bf16(wq_p.T)
    wkT_full = _bf16(wk_p.T)
    wvT_full = _bf16(wv.T)
    woT_full = _bf16(wo.T)
    cs_half = pos_cos[0].T.astype(np.float32)  # [64, s]
    sn_half = pos_sin[0].T.astype(np.float32)
    csn = _bf16(np.concatenate([cs_half, cs_half, sn_half, -sn_half], axis=0))
    xT_all = _bf16(np.concatenate([x[b].T for b in range(x.shape[0])], axis=0))
    in_maps = []
    for c in range(N_CORES):
        g_in = np.ascontiguousarray(np.concatenate(
            [xT_all[c * XROWS:(c + 1) * XROWS],
             csn[c * CSROWS:(c + 1) * CSROWS]], axis=0))
        in_maps.append({
            "g_in": g_in,
            "wqT": np.ascontiguousarray(wqT_full[:, c * lf:(c + 1) * lf]),
            "wkT": np.ascontiguousarray(wkT_full[:, c * lf:(c + 1) * lf]),
            "wvT": np.ascontiguousarray(wvT_full[:, c * lf:(c + 1) * lf]),
            "woT": np.ascontiguousarray(woT_full[c * lf:(c + 1) * lf, :]),
        })
    return in_maps


_NC_CACHE = {}


def _get_nc(s=S):
    if s not in _NC_CACHE:
        _NC_CACHE[s] = build_nc(s)
    return _NC_CACHE[s]


def _np_rope(t, cos, sin):
    b, ss, hh, hd = t.shape
    tr = t.reshape(b, ss, hh, hd // 2, 2)
    te, to = tr[..., 0], tr[..., 1]
    c = cos[:, :, None, :]
    s = sin[:, :, None, :]
    return np.stack([te * c - to * s, te * s + to * c], axis=-1).reshape(b, ss, hh, hd)


def _score_sample_max(x, wq, wk, pos_cos, pos_sin):
    """Sampled estimate of max |score|; the device softmax skips the max
    subtraction, which is only safe when scores stay well under exp's fp32
    range."""
    ss = x[:, :: max(1, x.shape[1] // 32), :][:, :32]
    pos_idx = np.arange(x.shape[1])[:: max(1, x.shape[1] // 32)][:32]
    h = x.shape[2] // HD
    q = (ss @ wq.T).reshape(ss.shape[0], -1, h, HD)
    k = (ss @ wk.T).reshape(ss.shape[0], -1, h, HD)
    c = pos_cos[:, pos_idx]
    sn = pos_sin[:, pos_idx]
    q = _np_rope(q, c, sn)
    k = _np_rope(k, c, sn)
    sc = np.einsum('bqhd,bkhd->bhqk', q, k) / math.sqrt(HD)
    return float(np.abs(sc).max())


def _np_fallback(x, wq, wk, wv, wo, pos_cos, pos_sin):
    out = np.empty_like(x)
    h = x.shape[2] // HD
    for b in range(x.shape[0]):
        q = _np_rope((x[b:b + 1] @ wq.T).reshape(1, -1, h, HD), pos_cos, pos_sin)
        k = _np_rope((x[b:b + 1] @ wk.T).reshape(1, -1, h, HD), pos_cos, pos_sin)
        v = (x[b:b + 1] @ wv.T).reshape(1, -1, h, HD)
        sc = np.einsum('bqhd,bkhd->bhqk', q, k) / math.sqrt(HD)
        sc -= sc.max(axis=-1, keepdims=True)
        e = np.exp(sc, dtype=np.float32)
        p = e / e.sum(axis=-1, keepdims=True)
        out[b] = (np.einsum('bhqk,bkhd->bqhd', p, v).reshape(1, x.shape[1], -1)
                  @ wo.T)[0]
    return out


def kernel(x, wq, wk, wv, wo, pos_cos, pos_sin):
    x = np.asarray(x, dtype=np.float32)
    wq, wk, wv, wo = (np.asarray(a, dtype=np.float32) for a in (wq, wk, wv, wo))
    pos_cos = np.asarray(pos_cos, dtype=np.float32)
    pos_sin = np.asarray(pos_sin, dtype=np.float32)
    # the device softmax skips max subtraction (safe for scores ~ N(0,1));
    # if the inputs are scaled such that exp would overflow, fall back to a
    # correct (slower) host path rather than returning inf/NaN
    if 4.0 * _score_sample_max(x, wq, wk, pos_cos, pos_sin) > 80.0:
        return _np_fallback(x, wq, wk, wv, wo, pos_cos, pos_sin)
    s = x.shape[1]
    in_maps = _prep_in_maps(x, wq, wk, wv, wo, pos_cos, pos_sin, s=s)
    nc = _get_nc(s)
    res = run_bass_kernel_spmd(nc, in_maps, core_ids=list(range(N_CORES)))
    yb = np.concatenate([res.results[c]["y"] for c in range(N_CORES)], axis=0)
    return yb.astype(np.float32).reshape(B, s, D)


# revision 3
# speedup vs baseline: 1.2341x; 1.2341x over previous
"""Multi-head attention (RoPE, softmax, out-proj) on 8 Trainium2 NeuronCores.

The tunnel between host and the axon-attached devices runs at ~40 MB/s, so
the wall time of run_bass_kernel_spmd is dominated by bytes crossing it, not
by device compute. This kernel is organized so every byte crosses exactly
once, in bf16:

  - tensor-parallel over all 8 cores: core c owns heads {2c, 2c+1}
    (column-parallel wq/wk/wv, row-parallel wo), and processes BOTH batches
    for those heads. Weight slices are disjoint across cores (no duplicate
    upload).
  - x (transposed, both batches stacked: [4096, S]) and the RoPE cos/sin
    rows ([256, S]) are sharded row-wise 8 ways, packed into one [544, S]
    bf16 tensor per core, and AllGathered on device (on-chip collective,
    ~70us) instead of being replicated over the tunnel.
  - each core's partial out-projection ([2S, D] f32) is ReduceScattered
    (add) across the 8 cores; each core converts its [2S/8, D] shard to
    bf16 and returns only that. The host concatenates the shards.

Compute structure per core is the proven head-group pipeline (matmuls in
bf16 at full PE rate with fp32 PSUM accumulation; RoPE as a partition-block
half-swap with host-permuted q/k feature rows and [+sin;-sin] sign folding;
softmax unnormalized in exp with the denominator reduced by an fp32r
ones-matmul and applied as a reciprocal multiply).
"""
import math
import sys

import numpy as np

for _p in ('/opt/trn_rl_repo', '/root/.axon_site/_ro/trn_rl_repo'):
    if _p not in sys.path:
        sys.path.insert(0, _p)

import ml_dtypes
import orjson

import concourse.bass as bass
import concourse.mybir as mybir
from concourse.tile import TileContext
from concourse.bass_utils import run_bass_kernel_spmd

F32 = mybir.dt.float32
R32 = mybir.dt.float32r
BF16 = mybir.dt.bfloat16
NP_BF16 = ml_dtypes.bfloat16

B = 2
S = 2048
D = 2048
HD = 128
N_HEADS = D // HD   # 16
N_CORES = 8
HPC = N_HEADS // N_CORES   # heads per core (2)
LF = HPC * HD              # local features per core (256)
XROWS = B * D // N_CORES   # x-shard rows per core (512)
CSROWS = 2 * HD // N_CORES  # cos/sin shard rows per core (32)
GROWS = XROWS + CSROWS     # packed gather-input rows (544)


# ---------------------------------------------------------------------------
# Wait-splitting post-pass: this toolchain's walrus supports at most ONE sync
# wait command per instruction (none at all on fp32/fp32r Matmult, which
# lowers to an LDW+MM pair). Tile emits multi-wait instructions; hoist the
# excess onto NoOps on the same engine immediately before the instruction.
# ---------------------------------------------------------------------------

def _keep_count(ins):
    if ins.get('opcode') == 'Matmult':
        dt = None
        for arg in ins.get('ins', []):
            dt = arg.get('dtype') or dt
        if dt in ('float32', 'float32r'):
            return 0
        return 1
    return 1


def _split_waits_json(data: bytes) -> bytes:
    d = orjson.loads(data)
    ctr = 0
    for fn in d.get('functions', []):
        for bb in fn.get('blocks', []):
            out = []
            for ins in bb.get('instructions', []):
                si = ins.get('sync_info')
                waits = (si or {}).get('on_wait') or []
                keep = _keep_count(ins)
                if len(waits) > keep:
                    hoist = waits[:len(waits) - keep]
                    keep_w = waits[len(waits) - keep:]
                    for w in hoist:
                        ctr += 1
                        nop = {
                            'name': f"{ins['name']}-ws{ctr}",
                            'opcode': 'NoOp',
                            'engine': ins.get('engine'),
                            'ins': [],
                            'outs': [],
                            'sync_info': {'on_wait': [w], 'on_update': []},
                        }
                        if 'debug' in ins:
                            nop['debug'] = ins['debug']
                        out.append(nop)
                    si['on_wait'] = keep_w
                out.append(ins)
            bb['instructions'] = out
    return orjson.dumps(d)


def _install_waitsplit():
    if getattr(bass.Bass, '_waitsplit_installed', False):
        return
    orig = bass.Bass.to_json_bytes

    def patched(self, *a, **k):
        # memoized: the module is immutable once built, and the per-call jit
        # lowering re-serializes it otherwise (~0.1s each dispatch)
        if a or k:
            return _split_waits_json(orig(self, *a, **k))
        cached = getattr(self, '_json_bytes_cache', None)
        if cached is None:
            cached = _split_waits_json(orig(self))
            self._json_bytes_cache = cached
        return cached

    bass.Bass.to_json_bytes = patched
    bass.Bass._waitsplit_installed = True


_install_waitsplit()


# ---------------------------------------------------------------------------
# Device program (SPMD, identical on all cores; per-core data differs)
# ---------------------------------------------------------------------------

def build_nc(s=S):
    d = D
    lf = LF
    hpc = HPC
    kd_n = d // 128          # contraction chunks for projections (16)
    nw = 512 if s >= 512 else s  # free-dim width per matmul
    nsq = s // nw            # wide column chunks
    ns = s // 128            # 128-row chunks
    nj = d // 512
    jw = 512
    ry = B * s // N_CORES    # output rows per core after reduce-scatter
    scale = 1.0 / math.sqrt(HD)
    rg = [list(range(N_CORES))]

    nc = bass.Bass()
    g_in = nc.dram_tensor("g_in", [GROWS, s], BF16, kind="ExternalInput")
    wqT = nc.dram_tensor("wqT", [d, lf], BF16, kind="ExternalInput")
    wkT = nc.dram_tensor("wkT", [d, lf], BF16, kind="ExternalInput")
    wvT = nc.dram_tensor("wvT", [d, lf], BF16, kind="ExternalInput")
    woT = nc.dram_tensor("woT", [lf, d], BF16, kind="ExternalInput")
    y = nc.dram_tensor("y", [ry, d], BF16, kind="ExternalOutput")

    def g_row(b, kd):
        # row in the gathered tensor of xT_all row b*D + kd*128
        a = b * d + kd * 128
        r, off = divmod(a, XROWS)
        return GROWS * r + off

    def cs_row(i):
        # row in the gathered tensor of csn_all row i (0:128 cos, 128:256 sin)
        r, off = divmod(i, CSROWS)
        return GROWS * r + XROWS + off

    with TileContext(nc) as tc:
        with tc.tile_pool(name="dram", bufs=1, space="DRAM") as dpool:
            g_bounce = dpool.tile([GROWS, s], BF16, name="g_bounce")
            gathered = dpool.tile([N_CORES * GROWS, s], BF16,
                                  addr_space="Shared", name="gathered")
            y_part = dpool.tile([B * s, d], F32, name="y_part")
            y_rs = dpool.tile([ry, d], F32, name="y_rs")

            nc.gpsimd.dma_start(out=g_bounce[:], in_=g_in[:])
            nc.gpsimd.collective_compute(
                "AllGather", mybir.AluOpType.bypass, replica_groups=rg,
                ins=[g_bounce.opt()], outs=[gathered.opt()])

            # Persistent SBUF residents: post-RoPE q/k and v for both batches
            # (4 virtual head-groups = 2 heads x 2 batches), and the fp32r
            # ones column used for the softmax denominator.
            with tc.tile_pool(name="persist", bufs=1) as per:
                qT_all = per.tile([128, B * hpc * s], BF16, name="qT_all")
                kT_all = per.tile([128, B * hpc * s], BF16, name="kT_all")
                v_all = per.tile([128, B * ns * lf], BF16, name="v_all")
                ones_f = per.tile([128, 128], F32, name="ones_f")
                nc.vector.memset(ones_f, 1.0)
                ones = per.tile([128, 128], R32, name="ones")
                nc.vector.tensor_copy(ones, ones_f)

                # ---------- Stage A: q/k/v projections + RoPE ----------
                with tc.tile_pool(name="wqk", bufs=1) as wpool, \
                     tc.tile_pool(name="xa", bufs=2) as xpool, \
                     tc.tile_pool(name="csp", bufs=1) as cspool, \
                     tc.tile_pool(name="rp", bufs=2) as rpool, \
                     tc.tile_pool(name="psA", bufs=4, space="PSUM") as pspool, \
                     tc.tile_pool(name="psAV", bufs=2, space="PSUM") as pvpool:
                    wq_sb = wpool.tile([128, kd_n * lf], BF16, name="wq_sb")
                    wk_sb = wpool.tile([128, kd_n * lf], BF16, name="wk_sb")
                    wv_sb = wpool.tile([128, kd_n * lf], BF16, name="wv_sb")
                    for kd in range(kd_n):
                        nc.sync.dma_start(out=wq_sb[:, kd * lf:(kd + 1) * lf],
                                          in_=wqT[kd * 128:(kd + 1) * 128, :])
                        nc.scalar.dma_start(out=wk_sb[:, kd * lf:(kd + 1) * lf],
                                            in_=wkT[kd * 128:(kd + 1) * 128, :])
                        nc.scalar.dma_start(out=wv_sb[:, kd * lf:(kd + 1) * lf],
                                            in_=wvT[kd * 128:(kd + 1) * 128, :])

                    # cos/sin: gathered bf16 rows -> SBUF -> f32. sn rows are
                    # [+sin; -sin] (host-prepared) so the half-swap cross
                    # terms land with the right signs.
                    cs_bf = cspool.tile([128, s], BF16, name="cs_bf")
                    sn_bf = cspool.tile([128, s], BF16, name="sn_bf")
                    for i in range(0, 128, CSROWS):
                        nc.sync.dma_start(
                            out=cs_bf[i:i + CSROWS, :],
                            in_=gathered[cs_row(i):cs_row(i) + CSROWS, :])
                        nc.sync.dma_start(
                            out=sn_bf[i:i + CSROWS, :],
                            in_=gathered[cs_row(128 + i):cs_row(128 + i) + CSROWS, :])
                    cs_sb = cspool.tile([128, s], F32, name="cs_sb")
                    sn_sb = cspool.tile([128, s], F32, name="sn_sb")
                    nc.vector.tensor_copy(cs_sb, cs_bf)
                    nc.vector.tensor_copy(sn_sb, sn_bf)

                    def load_x(b, sq):
                        t = xpool.tile([128, kd_n * nw], BF16, name="x_sb")
                        for kd in range(kd_n):
                            r = g_row(b, kd)
                            nc.sync.dma_start(
                                out=t[:, kd * nw:(kd + 1) * nw],
                                in_=gathered[r:r + 128, sq * nw:(sq + 1) * nw])
                        return t

                    def emit_v(b, sq, x_tile):
                        for ss in range(nw // 128):
                            psv = pvpool.tile([128, lf], F32, name="psv")
                            for kd in range(kd_n):
                                nc.tensor.matmul(
                                    psv,
                                    x_tile[:, kd * nw + ss * 128:
                                           kd * nw + (ss + 1) * 128],
                                    wv_sb[:, kd * lf:(kd + 1) * lf],
                                    start=(kd == 0), stop=(kd == kd_n - 1))
                            sk = sq * (nw // 128) + ss
                            nc.vector.tensor_copy(
                                v_all[:, (b * ns + sk) * lf:(b * ns + sk + 1) * lf],
                                psv)

                    x_prev = None
                    x_next = load_x(0, 0)
                    for bi in range(B):
                        for sq in range(nsq):
                            x_sb = x_next
                            if not (bi == B - 1 and sq == nsq - 1):
                                nb, nq = (bi, sq + 1) if sq + 1 < nsq else (bi + 1, 0)
                                x_next = load_x(nb, nq)
                            for wsb, dstT in ((wq_sb, qT_all), (wk_sb, kT_all)):
                                for h in range(hpc):
                                    g4 = bi * hpc + h
                                    ps = pspool.tile([128, nw], F32, name="ps_qk")
                                    for kd in range(kd_n):
                                        nc.tensor.matmul(
                                            ps,
                                            wsb[:, kd * lf + h * 128:
                                                kd * lf + (h + 1) * 128],
                                            x_sb[:, kd * nw:(kd + 1) * nw],
                                            start=(kd == 0), stop=(kd == kd_n - 1))
                                    tcc = rpool.tile([128, nw], F32, name="t_c")
                                    tss = rpool.tile([128, nw], F32, name="t_s")
                                    nc.vector.tensor_mul(
                                        tcc, ps, cs_sb[:, sq * nw:(sq + 1) * nw])
                                    nc.vector.tensor_mul(
                                        tss, ps, sn_sb[:, sq * nw:(sq + 1) * nw])
                                    tsw = rpool.tile([128, nw], F32, name="t_sw")
                                    nc.sync.dma_start(out=tsw[0:64, :],
                                                      in_=tss[64:128, :])
                                    nc.sync.dma_start(out=tsw[64:128, :],
                                                      in_=tss[0:64, :])
                                    nc.vector.tensor_add(
                                        dstT[:, g4 * s + sq * nw:
                                             g4 * s + sq * nw + nw], tcc, tsw)
                            if x_prev is not None:
                                pb, pq, pt = x_prev
                                emit_v(pb, pq, pt)
                            x_prev = (bi, sq, x_sb)
                    pb, pq, pt = x_prev
                    emit_v(pb, pq, pt)

                # ---------- Stage B+C: attention, then partial out-proj ----------
                with tc.tile_pool(name="exp", bufs=2) as expool, \
                     tc.tile_pool(name="nrm", bufs=2) as npool, \
                     tc.tile_pool(name="atp", bufs=2) as atpool, \
                     tc.tile_pool(name="wop", bufs=1) as wopool, \
                     tc.tile_pool(name="yop", bufs=3) as yopool, \
                     tc.tile_pool(name="psS", bufs=3, space="PSUM") as pssc, \
                     tc.tile_pool(name="psM", bufs=1, space="PSUM") as pssm, \
                     tc.tile_pool(name="psV", bufs=2, space="PSUM") as psov, \
                     tc.tile_pool(name="psC", bufs=2, space="PSUM") as psc:
                    wo_sb = wopool.tile([128, hpc * d], BF16, name="wo_sb")
                    for i in range(hpc):
                        nc.sync.dma_start(out=wo_sb[:, i * d:(i + 1) * d],
                                          in_=woT[i * 128:(i + 1) * 128, :])
                    nsub = nw // 128

                    def emit_c_part(bq, aT_tile, ssub):
                        # one query-row slice of the partial out-projection
                        bi, sq = bq
                        for jn in range(nj):
                            yps = psc.tile([128, jw], F32, name="yps")
                            for i in range(hpc):
                                nc.tensor.matmul(
                                    yps,
                                    aT_tile[:, i * nw + ssub * 128:
                                            i * nw + (ssub + 1) * 128],
                                    wo_sb[:, i * d + jn * jw:(i * d + (jn + 1) * jw)],
                                    start=(i == 0), stop=(i == hpc - 1))
                            yo = yopool.tile([128, jw], F32, name="yo")
                            nc.vector.tensor_copy(yo, yps)
                            r0 = bi * s + sq * nw + ssub * 128
                            nc.sync.dma_start(
                                out=y_part[r0:r0 + 128, jn * jw:(jn + 1) * jw],
                                in_=yo)

                    prev_c = None  # ((bi, sq), aT_tile) of the previous chunk
                    for bi in range(B):
                        for sq in range(nsq):
                            aT_sq = atpool.tile([128, hpc * nw], BF16, name="aT_sq")
                            for h in range(hpc):
                                g4 = bi * hpc + h
                                qT_sl = qT_all[:, g4 * s + sq * nw:
                                               g4 * s + (sq + 1) * nw]
                                ex_sb = expool.tile([128, ns * nw], BF16, name="ex_sb")
                                acc = npool.tile([128, nw], F32, name="acc")
                                pairs = []
                                for sk in range(ns):
                                    sps = pssc.tile([128, nw], F32, name="sps")
                                    nc.tensor.matmul(
                                        sps,
                                        kT_all[:, g4 * s + sk * 128:
                                               g4 * s + (sk + 1) * 128],
                                        qT_sl, start=True, stop=True)
                                    nc.scalar.activation(
                                        ex_sb[:, sk * nw:(sk + 1) * nw], sps,
                                        mybir.ActivationFunctionType.Exp,
                                        scale=scale)
                                    # pairwise level-0 exp sums on the
                                    # otherwise-idle GPSIMD engine
                                    if sk % 2 == 1:
                                        pr = npool.tile([128, nw], F32,
                                                        name=f"pr{sk // 2}")
                                        nc.gpsimd.tensor_add(
                                            pr, ex_sb[:, (sk - 1) * nw:sk * nw],
                                            ex_sb[:, sk * nw:(sk + 1) * nw])
                                        pairs.append(pr)
                                if ns == 1:
                                    nc.vector.tensor_copy(acc, ex_sb[:, 0:nw])
                                else:
                                    nc.vector.tensor_add(acc, pairs[0], pairs[1])
                                    for pr in pairs[2:]:
                                        nc.vector.tensor_add(acc, acc, pr)
                                ov = psov.tile([128, nw], F32, name="ov")
                                for sk in range(ns):
                                    nc.tensor.matmul(
                                        ov,
                                        v_all[:, (bi * ns + sk) * lf + h * 128:
                                              (bi * ns + sk) * lf + (h + 1) * 128],
                                        ex_sb[:, sk * nw:(sk + 1) * nw],
                                        start=(sk == 0), stop=(sk == ns - 1))
                                accr = npool.tile([128, nw], R32, name="accr")
                                nc.vector.tensor_copy(accr, acc)
                                # partition reduction + row broadcast of the
                                # softmax denominator
                                sm = pssm.tile([128, nw], F32, name="sm")
                                nc.tensor.matmul(sm, ones, accr, start=True,
                                                 stop=True)
                                rec = npool.tile([128, nw], F32, name="rec")
                                nc.vector.reciprocal(rec, sm)
                                nc.vector.tensor_mul(
                                    aT_sq[:, h * nw:(h + 1) * nw], ov, rec)
                                # interleave the PREVIOUS chunk's out-projection
                                # slices between heads
                                if prev_c is not None:
                                    pbq, pat = prev_c
                                    lo = h * nsub // hpc
                                    hi = (h + 1) * nsub // hpc
                                    for ssub in range(lo, hi):
                                        emit_c_part(pbq, pat, ssub)
                            prev_c = ((bi, sq), aT_sq)
                    pbq, pat = prev_c
                    for ssub in range(nsub):
                        emit_c_part(pbq, pat, ssub)

                # ---------- Stage D: reduce-scatter + bf16 convert ----------
                with tc.tile_pool(name="cvt", bufs=2) as cpool:
                    nc.gpsimd.collective_compute(
                        "ReduceScatter", mybir.AluOpType.add, replica_groups=rg,
                        ins=[y_part.opt()], outs=[y_rs.opt()])
                    for r0 in range(0, ry, 128):
                        yf = cpool.tile([128, d], F32, name="yf")
                        nc.sync.dma_start(out=yf, in_=y_rs[r0:r0 + 128, :])
                        yb = cpool.tile([128, d], BF16, name="yb")
                        nc.vector.tensor_copy(yb, yf)
                        nc.sync.dma_start(out=y[r0:r0 + 128, :], in_=yb)
    return nc


# ---------------------------------------------------------------------------
# Host-side prep + gather
# ---------------------------------------------------------------------------

_PERM_HEAD = np.concatenate([np.arange(0, HD, 2), np.arange(1, HD, 2)])


def _bf16(a):
    """Fast float32 -> bfloat16 with round-to-nearest-even (bit twiddling —
    ~4x faster than ml_dtypes astype on large arrays)."""
    a = np.ascontiguousarray(a, dtype=np.float32)
    u = a.view(np.uint32)
    r = ((u >> 16) & 1) + np.uint32(0x7FFF)
    return ((u + r) >> 16).astype(np.uint16).view(NP_BF16).reshape(a.shape)


def _prep_in_maps(x, wq, wk, wv, wo, pos_cos, pos_sin, s=S):
    d = D
    lf = LF
    # permute q/k feature rows within each head: even pairs first, then odd
    wq_p = wq.reshape(N_HEADS, HD, d)[:, _PERM_HEAD, :].reshape(d, d)
    wk_p = wk.reshape(N_HEADS, HD, d)[:, _PERM_HEAD, :].reshape(d, d)
    wqT_full = _bf16(wq_p.T)
    wkT_full = _bf16(wk_p.T)
    wvT_full = _bf16(wv.T)
    woT_full = _bf16(wo.T)
    cs_half = pos_cos[0].T.astype(np.float32)  # [64, s]
    sn_half = pos_sin[0].T.astype(np.float32)
    csn = _bf16(np.concatenate([cs_half, cs_half, sn_half, -sn_half], axis=0))
    xT_all = _bf16(np.concatenate([x[b].T for b in range(x.shape[0])], axis=0))
    in_maps = []
    for c in range(N_CORES):
        g_in = np.ascontiguousarray(np.concatenate(
            [xT_all[c * XROWS:(c + 1) * XROWS],
             csn[c * CSROWS:(c + 1) * CSROWS]], axis=0))
        in_maps.append({
            "g_in": g_in,
            "wqT": np.ascontiguousarray(wqT_full[:, c * lf:(c + 1) * lf]),
            "wkT": np.ascontiguousarray(wkT_full[:, c * lf:(c + 1) * lf]),
            "wvT": np.ascontiguousarray(wvT_full[:, c * lf:(c + 1) * lf]),
            "woT": np.ascontiguousarray(woT_full[c * lf:(c + 1) * lf, :]),
        })
    return in_maps


_NC_CACHE = {}


def _get_nc(s=S):
    if s not in _NC_CACHE:
        _NC_CACHE[s] = build_nc(s)
    return _NC_CACHE[s]


def _np_rope(t, cos, sin):
    b, ss, hh, hd = t.shape
    tr = t.reshape(b, ss, hh, hd // 2, 2)
    te, to = tr[..., 0], tr[..., 1]
    c = cos[:, :, None, :]
    s = sin[:, :, None, :]
    return np.stack([te * c - to * s, te * s + to * c], axis=-1).reshape(b, ss, hh, hd)


def _score_sample_max(x, wq, wk, pos_cos, pos_sin):
    """Sampled estimate of max |score|; the device softmax skips the max
    subtraction, which is only safe when scores stay well under exp's fp32
    range."""
    ss = x[:, :: max(1, x.shape[1] // 32), :][:, :32]
    pos_idx = np.arange(x.shape[1])[:: max(1, x.shape[1] // 32)][:32]
    h = x.shape[2] // HD
    q = (ss @ wq.T).reshape(ss.shape[0], -1, h, HD)
    k = (ss @ wk.T).reshape(ss.shape[0], -1, h, HD)
    c = pos_cos[:, pos_idx]
    sn = pos_sin[:, pos_idx]
    q = _np_rope(q, c, sn)
    k = _np_rope(k, c, sn)
    sc = np.einsum('bqhd,bkhd->bhqk', q, k) / math.sqrt(HD)
    return float(np.abs(sc).max())


def _np_fallback(x, wq, wk, wv, wo, pos_cos, pos_sin):
    out = np.empty_like(x)
    h = x.shape[2] // HD
    for b in range(x.shape[0]):
        q = _np_rope((x[b:b + 1] @ wq.T).reshape(1, -1, h, HD), pos_cos, pos_sin)
        k = _np_rope((x[b:b + 1] @ wk.T).reshape(1, -1, h, HD), pos_cos, pos_sin)
        v = (x[b:b + 1] @ wv.T).reshape(1, -1, h, HD)
        sc = np.einsum('bqhd,bkhd->bhqk', q, k) / math.sqrt(HD)
        sc -= sc.max(axis=-1, keepdims=True)
        e = np.exp(sc, dtype=np.float32)
        p = e / e.sum(axis=-1, keepdims=True)
        out[b] = (np.einsum('bhqk,bkhd->bqhd', p, v).reshape(1, x.shape[1], -1)
                  @ wo.T)[0]
    return out


def kernel(x, wq, wk, wv, wo, pos_cos, pos_sin):
    x = np.asarray(x, dtype=np.float32)
    wq, wk, wv, wo = (np.asarray(a, dtype=np.float32) for a in (wq, wk, wv, wo))
    pos_cos = np.asarray(pos_cos, dtype=np.float32)
    pos_sin = np.asarray(pos_sin, dtype=np.float32)
    # the device softmax skips max subtraction (safe for scores ~ N(0,1));
    # if the inputs are scaled such that exp would overflow, fall back to a
    # correct (slower) host path rather than returning inf/NaN
    if 4.0 * _score_sample_max(x, wq, wk, pos_cos, pos_sin) > 80.0:
        return _np_fallback(x, wq, wk, wv, wo, pos_cos, pos_sin)
    s = x.shape[1]
    in_maps = _prep_in_maps(x, wq, wk, wv, wo, pos_cos, pos_sin, s=s)
    nc = _get_nc(s)
    res = run_bass_kernel_spmd(nc, in_maps, core_ids=list(range(N_CORES)))
    yb = np.concatenate([res.results[c]["y"] for c in range(N_CORES)], axis=0)
    return yb.astype(np.float32).reshape(B, s, D)


# revision 4
# speedup vs baseline: 1.6140x; 1.3079x over previous
"""Multi-head attention (RoPE, softmax, out-proj) on 8 Trainium2 NeuronCores.

The tunnel between host and the axon-attached devices runs at ~40 MB/s, so
the wall time of run_bass_kernel_spmd is dominated by bytes crossing it, not
by device compute. This kernel is organized so every byte crosses exactly
once, in bf16:

  - tensor-parallel over all 8 cores: core c owns heads {2c, 2c+1}
    (column-parallel wq/wk/wv, row-parallel wo), and processes BOTH batches
    for those heads. Weight slices are disjoint across cores (no duplicate
    upload).
  - x (transposed, both batches stacked: [4096, S]) and the RoPE cos/sin
    rows ([256, S]) are sharded row-wise 8 ways, packed into one [544, S]
    bf16 tensor per core, and AllGathered on device (on-chip collective,
    ~70us) instead of being replicated over the tunnel.
  - each core's partial out-projection ([2S, D] f32) is ReduceScattered
    (add) across the 8 cores; each core converts its [2S/8, D] shard to
    bf16 and returns only that. The host concatenates the shards.

Compute structure per core is the proven head-group pipeline (matmuls in
bf16 at full PE rate with fp32 PSUM accumulation; RoPE as a partition-block
half-swap with host-permuted q/k feature rows and [+sin;-sin] sign folding;
softmax unnormalized in exp with the denominator reduced by an fp32r
ones-matmul and applied as a reciprocal multiply).
"""
import math
import sys

import numpy as np

for _p in ('/opt/trn_rl_repo', '/root/.axon_site/_ro/trn_rl_repo'):
    if _p not in sys.path:
        sys.path.insert(0, _p)

import ml_dtypes
import orjson

import concourse.bass as bass
import concourse.mybir as mybir
from concourse.tile import TileContext
from concourse.bass_utils import run_bass_kernel_spmd

# The dispatch path builds a fresh jax.jit per call; without the persistent
# compilation cache each call re-lowers and re-compiles the wrapped NEFF
# executable (~0.4s). With it, steady-state calls hit the disk cache.
try:
    import jax as _jax
    _jax.config.update("jax_compilation_cache_dir", "/tmp/jax_comp_cache")
    _jax.config.update("jax_persistent_cache_min_entry_size_bytes", 0)
    _jax.config.update("jax_persistent_cache_min_compile_time_secs", 0.0)
except Exception:
    pass

F32 = mybir.dt.float32
R32 = mybir.dt.float32r
BF16 = mybir.dt.bfloat16
NP_BF16 = ml_dtypes.bfloat16

B = 2
S = 2048
D = 2048
HD = 128
N_HEADS = D // HD   # 16
N_CORES = 8
HPC = N_HEADS // N_CORES   # heads per core (2)
LF = HPC * HD              # local features per core (256)
XROWS = B * D // N_CORES   # x-shard rows per core (512)
CSROWS = 2 * HD // N_CORES  # cos/sin shard rows per core (32)
GROWS = XROWS + CSROWS     # packed gather-input rows (544)


# ---------------------------------------------------------------------------
# Wait-splitting post-pass: this toolchain's walrus supports at most ONE sync
# wait command per instruction (none at all on fp32/fp32r Matmult, which
# lowers to an LDW+MM pair). Tile emits multi-wait instructions; hoist the
# excess onto NoOps on the same engine immediately before the instruction.
# ---------------------------------------------------------------------------

def _keep_count(ins):
    if ins.get('opcode') == 'Matmult':
        dt = None
        for arg in ins.get('ins', []):
            dt = arg.get('dtype') or dt
        if dt in ('float32', 'float32r'):
            return 0
        return 1
    return 1


def _split_waits_json(data: bytes) -> bytes:
    d = orjson.loads(data)
    ctr = 0
    for fn in d.get('functions', []):
        for bb in fn.get('blocks', []):
            out = []
            for ins in bb.get('instructions', []):
                si = ins.get('sync_info')
                waits = (si or {}).get('on_wait') or []
                keep = _keep_count(ins)
                if len(waits) > keep:
                    hoist = waits[:len(waits) - keep]
                    keep_w = waits[len(waits) - keep:]
                    for w in hoist:
                        ctr += 1
                        nop = {
                            'name': f"{ins['name']}-ws{ctr}",
                            'opcode': 'NoOp',
                            'engine': ins.get('engine'),
                            'ins': [],
                            'outs': [],
                            'sync_info': {'on_wait': [w], 'on_update': []},
                        }
                        if 'debug' in ins:
                            nop['debug'] = ins['debug']
                        out.append(nop)
                    si['on_wait'] = keep_w
                out.append(ins)
            bb['instructions'] = out
    return orjson.dumps(d)


def _install_waitsplit():
    if getattr(bass.Bass, '_waitsplit_installed', False):
        return
    orig = bass.Bass.to_json_bytes

    def patched(self, *a, **k):
        # memoized: the module is immutable once built, and the per-call jit
        # lowering re-serializes it otherwise (~0.1s each dispatch)
        if a or k:
            return _split_waits_json(orig(self, *a, **k))
        cached = getattr(self, '_json_bytes_cache', None)
        if cached is None:
            cached = _split_waits_json(orig(self))
            self._json_bytes_cache = cached
        return cached

    bass.Bass.to_json_bytes = patched
    bass.Bass._waitsplit_installed = True


_install_waitsplit()


# ---------------------------------------------------------------------------
# Device program (SPMD, identical on all cores; per-core data differs)
# ---------------------------------------------------------------------------

def build_nc(s=S):
    d = D
    lf = LF
    hpc = HPC
    kd_n = d // 128          # contraction chunks for projections (16)
    nw = 512 if s >= 512 else s  # free-dim width per matmul
    nsq = s // nw            # wide column chunks
    ns = s // 128            # 128-row chunks
    nj = d // 512
    jw = 512
    ry = B * s // N_CORES    # output rows per core after reduce-scatter
    scale = 1.0 / math.sqrt(HD)
    rg = [list(range(N_CORES))]

    nc = bass.Bass()
    g_in = nc.dram_tensor("g_in", [GROWS, s], BF16, kind="ExternalInput")
    wqT = nc.dram_tensor("wqT", [d, lf], BF16, kind="ExternalInput")
    wkT = nc.dram_tensor("wkT", [d, lf], BF16, kind="ExternalInput")
    wvT = nc.dram_tensor("wvT", [d, lf], BF16, kind="ExternalInput")
    woT = nc.dram_tensor("woT", [lf, d], BF16, kind="ExternalInput")
    y = nc.dram_tensor("y", [ry, d], BF16, kind="ExternalOutput")

    def g_row(b, kd):
        # row in the gathered tensor of xT_all row b*D + kd*128
        a = b * d + kd * 128
        r, off = divmod(a, XROWS)
        return GROWS * r + off

    def cs_row(i):
        # row in the gathered tensor of csn_all row i (0:128 cos, 128:256 sin)
        r, off = divmod(i, CSROWS)
        return GROWS * r + XROWS + off

    with TileContext(nc) as tc:
        with tc.tile_pool(name="dram", bufs=1, space="DRAM") as dpool:
            g_bounce = dpool.tile([GROWS, s], BF16, name="g_bounce")
            gathered = dpool.tile([N_CORES * GROWS, s], BF16,
                                  addr_space="Shared", name="gathered")
            y_part = dpool.tile([B * s, d], F32, name="y_part")
            y_rs = dpool.tile([ry, d], F32, name="y_rs")

            nc.gpsimd.dma_start(out=g_bounce[:], in_=g_in[:])
            nc.gpsimd.collective_compute(
                "AllGather", mybir.AluOpType.bypass, replica_groups=rg,
                ins=[g_bounce.opt()], outs=[gathered.opt()])

            # Persistent SBUF residents: post-RoPE q/k and v for both batches
            # (4 virtual head-groups = 2 heads x 2 batches), and the fp32r
            # ones column used for the softmax denominator.
            with tc.tile_pool(name="persist", bufs=1) as per:
                qT_all = per.tile([128, B * hpc * s], BF16, name="qT_all")
                kT_all = per.tile([128, B * hpc * s], BF16, name="kT_all")
                v_all = per.tile([128, B * ns * lf], BF16, name="v_all")
                ones_f = per.tile([128, 128], F32, name="ones_f")
                nc.vector.memset(ones_f, 1.0)
                ones = per.tile([128, 128], R32, name="ones")
                nc.vector.tensor_copy(ones, ones_f)

                # ---------- Stage A: q/k/v projections + RoPE ----------
                with tc.tile_pool(name="wqk", bufs=1) as wpool, \
                     tc.tile_pool(name="xa", bufs=2) as xpool, \
                     tc.tile_pool(name="csp", bufs=1) as cspool, \
                     tc.tile_pool(name="rp", bufs=2) as rpool, \
                     tc.tile_pool(name="psA", bufs=4, space="PSUM") as pspool, \
                     tc.tile_pool(name="psAV", bufs=2, space="PSUM") as pvpool:
                    wq_sb = wpool.tile([128, kd_n * lf], BF16, name="wq_sb")
                    wk_sb = wpool.tile([128, kd_n * lf], BF16, name="wk_sb")
                    wv_sb = wpool.tile([128, kd_n * lf], BF16, name="wv_sb")
                    for kd in range(kd_n):
                        nc.sync.dma_start(out=wq_sb[:, kd * lf:(kd + 1) * lf],
                                          in_=wqT[kd * 128:(kd + 1) * 128, :])
                        nc.scalar.dma_start(out=wk_sb[:, kd * lf:(kd + 1) * lf],
                                            in_=wkT[kd * 128:(kd + 1) * 128, :])
                        nc.scalar.dma_start(out=wv_sb[:, kd * lf:(kd + 1) * lf],
                                            in_=wvT[kd * 128:(kd + 1) * 128, :])

                    # cos/sin: gathered bf16 rows -> SBUF -> f32. sn rows are
                    # [+sin; -sin] (host-prepared) so the half-swap cross
                    # terms land with the right signs.
                    cs_bf = cspool.tile([128, s], BF16, name="cs_bf")
                    sn_bf = cspool.tile([128, s], BF16, name="sn_bf")
                    for i in range(0, 128, CSROWS):
                        nc.sync.dma_start(
                            out=cs_bf[i:i + CSROWS, :],
                            in_=gathered[cs_row(i):cs_row(i) + CSROWS, :])
                        nc.sync.dma_start(
                            out=sn_bf[i:i + CSROWS, :],
                            in_=gathered[cs_row(128 + i):cs_row(128 + i) + CSROWS, :])
                    cs_sb = cspool.tile([128, s], F32, name="cs_sb")
                    sn_sb = cspool.tile([128, s], F32, name="sn_sb")
                    nc.vector.tensor_copy(cs_sb, cs_bf)
                    nc.vector.tensor_copy(sn_sb, sn_bf)

                    def load_x(b, sq):
                        t = xpool.tile([128, kd_n * nw], BF16, name="x_sb")
                        for kd in range(kd_n):
                            r = g_row(b, kd)
                            nc.sync.dma_start(
                                out=t[:, kd * nw:(kd + 1) * nw],
                                in_=gathered[r:r + 128, sq * nw:(sq + 1) * nw])
                        return t

                    def emit_v(b, sq, x_tile):
                        for ss in range(nw // 128):
                            psv = pvpool.tile([128, lf], F32, name="psv")
                            for kd in range(kd_n):
                                nc.tensor.matmul(
                                    psv,
                                    x_tile[:, kd * nw + ss * 128:
                                           kd * nw + (ss + 1) * 128],
                                    wv_sb[:, kd * lf:(kd + 1) * lf],
                                    start=(kd == 0), stop=(kd == kd_n - 1))
                            sk = sq * (nw // 128) + ss
                            nc.vector.tensor_copy(
                                v_all[:, (b * ns + sk) * lf:(b * ns + sk + 1) * lf],
                                psv)

                    x_prev = None
                    x_next = load_x(0, 0)
                    for bi in range(B):
                        for sq in range(nsq):
                            x_sb = x_next
                            if not (bi == B - 1 and sq == nsq - 1):
                                nb, nq = (bi, sq + 1) if sq + 1 < nsq else (bi + 1, 0)
                                x_next = load_x(nb, nq)
                            for wsb, dstT in ((wq_sb, qT_all), (wk_sb, kT_all)):
                                for h in range(hpc):
                                    g4 = bi * hpc + h
                                    ps = pspool.tile([128, nw], F32, name="ps_qk")
                                    for kd in range(kd_n):
                                        nc.tensor.matmul(
                                            ps,
                                            wsb[:, kd * lf + h * 128:
                                                kd * lf + (h + 1) * 128],
                                            x_sb[:, kd * nw:(kd + 1) * nw],
                                            start=(kd == 0), stop=(kd == kd_n - 1))
                                    tcc = rpool.tile([128, nw], F32, name="t_c")
                                    tss = rpool.tile([128, nw], F32, name="t_s")
                                    nc.vector.tensor_mul(
                                        tcc, ps, cs_sb[:, sq * nw:(sq + 1) * nw])
                                    nc.vector.tensor_mul(
                                        tss, ps, sn_sb[:, sq * nw:(sq + 1) * nw])
                                    tsw = rpool.tile([128, nw], F32, name="t_sw")
                                    nc.sync.dma_start(out=tsw[0:64, :],
                                                      in_=tss[64:128, :])
                                    nc.sync.dma_start(out=tsw[64:128, :],
                                                      in_=tss[0:64, :])
                                    nc.vector.tensor_add(
                                        dstT[:, g4 * s + sq * nw:
                                             g4 * s + sq * nw + nw], tcc, tsw)
                            if x_prev is not None:
                                pb, pq, pt = x_prev
                                emit_v(pb, pq, pt)
                            x_prev = (bi, sq, x_sb)
                    pb, pq, pt = x_prev
                    emit_v(pb, pq, pt)

                # ---------- Stage B+C: attention, then partial out-proj ----------
                with tc.tile_pool(name="exp", bufs=2) as expool, \
                     tc.tile_pool(name="nrm", bufs=2) as npool, \
                     tc.tile_pool(name="atp", bufs=2) as atpool, \
                     tc.tile_pool(name="wop", bufs=1) as wopool, \
                     tc.tile_pool(name="yop", bufs=3) as yopool, \
                     tc.tile_pool(name="psS", bufs=3, space="PSUM") as pssc, \
                     tc.tile_pool(name="psM", bufs=1, space="PSUM") as pssm, \
                     tc.tile_pool(name="psV", bufs=2, space="PSUM") as psov, \
                     tc.tile_pool(name="psC", bufs=2, space="PSUM") as psc:
                    wo_sb = wopool.tile([128, hpc * d], BF16, name="wo_sb")
                    for i in range(hpc):
                        nc.sync.dma_start(out=wo_sb[:, i * d:(i + 1) * d],
                                          in_=woT[i * 128:(i + 1) * 128, :])
                    nsub = nw // 128

                    def emit_c_part(bq, aT_tile, ssub):
                        # one query-row slice of the partial out-projection
                        bi, sq = bq
                        for jn in range(nj):
                            yps = psc.tile([128, jw], F32, name="yps")
                            for i in range(hpc):
                                nc.tensor.matmul(
                                    yps,
                                    aT_tile[:, i * nw + ssub * 128:
                                            i * nw + (ssub + 1) * 128],
                                    wo_sb[:, i * d + jn * jw:(i * d + (jn + 1) * jw)],
                                    start=(i == 0), stop=(i == hpc - 1))
                            yo = yopool.tile([128, jw], F32, name="yo")
                            nc.vector.tensor_copy(yo, yps)
                            r0 = bi * s + sq * nw + ssub * 128
                            nc.sync.dma_start(
                                out=y_part[r0:r0 + 128, jn * jw:(jn + 1) * jw],
                                in_=yo)

                    prev_c = None  # ((bi, sq), aT_tile) of the previous chunk
                    for bi in range(B):
                        for sq in range(nsq):
                            aT_sq = atpool.tile([128, hpc * nw], BF16, name="aT_sq")
                            for h in range(hpc):
                                g4 = bi * hpc + h
                                qT_sl = qT_all[:, g4 * s + sq * nw:
                                               g4 * s + (sq + 1) * nw]
                                ex_sb = expool.tile([128, ns * nw], BF16, name="ex_sb")
                                acc = npool.tile([128, nw], F32, name="acc")
                                pairs = []
                                for sk in range(ns):
                                    sps = pssc.tile([128, nw], F32, name="sps")
                                    nc.tensor.matmul(
                                        sps,
                                        kT_all[:, g4 * s + sk * 128:
                                               g4 * s + (sk + 1) * 128],
                                        qT_sl, start=True, stop=True)
                                    nc.scalar.activation(
                                        ex_sb[:, sk * nw:(sk + 1) * nw], sps,
                                        mybir.ActivationFunctionType.Exp,
                                        scale=scale)
                                    # pairwise level-0 exp sums on the
                                    # otherwise-idle GPSIMD engine
                                    if sk % 2 == 1:
                                        pr = npool.tile([128, nw], F32,
                                                        name=f"pr{sk // 2}")
                                        nc.gpsimd.tensor_add(
                                            pr, ex_sb[:, (sk - 1) * nw:sk * nw],
                                            ex_sb[:, sk * nw:(sk + 1) * nw])
                                        pairs.append(pr)
                                if ns == 1:
                                    nc.vector.tensor_copy(acc, ex_sb[:, 0:nw])
                                else:
                                    nc.vector.tensor_add(acc, pairs[0], pairs[1])
                                    for pr in pairs[2:]:
                                        nc.vector.tensor_add(acc, acc, pr)
                                ov = psov.tile([128, nw], F32, name="ov")
                                for sk in range(ns):
                                    nc.tensor.matmul(
                                        ov,
                                        v_all[:, (bi * ns + sk) * lf + h * 128:
                                              (bi * ns + sk) * lf + (h + 1) * 128],
                                        ex_sb[:, sk * nw:(sk + 1) * nw],
                                        start=(sk == 0), stop=(sk == ns - 1))
                                accr = npool.tile([128, nw], R32, name="accr")
                                nc.vector.tensor_copy(accr, acc)
                                # partition reduction + row broadcast of the
                                # softmax denominator
                                sm = pssm.tile([128, nw], F32, name="sm")
                                nc.tensor.matmul(sm, ones, accr, start=True,
                                                 stop=True)
                                rec = npool.tile([128, nw], F32, name="rec")
                                nc.vector.reciprocal(rec, sm)
                                nc.vector.tensor_mul(
                                    aT_sq[:, h * nw:(h + 1) * nw], ov, rec)
                                # interleave the PREVIOUS chunk's out-projection
                                # slices between heads
                                if prev_c is not None:
                                    pbq, pat = prev_c
                                    lo = h * nsub // hpc
                                    hi = (h + 1) * nsub // hpc
                                    for ssub in range(lo, hi):
                                        emit_c_part(pbq, pat, ssub)
                            prev_c = ((bi, sq), aT_sq)
                    pbq, pat = prev_c
                    for ssub in range(nsub):
                        emit_c_part(pbq, pat, ssub)

                # ---------- Stage D: reduce-scatter + bf16 convert ----------
                with tc.tile_pool(name="cvt", bufs=2) as cpool:
                    nc.gpsimd.collective_compute(
                        "ReduceScatter", mybir.AluOpType.add, replica_groups=rg,
                        ins=[y_part.opt()], outs=[y_rs.opt()])
                    for r0 in range(0, ry, 128):
                        yf = cpool.tile([128, d], F32, name="yf")
                        nc.sync.dma_start(out=yf, in_=y_rs[r0:r0 + 128, :])
                        yb = cpool.tile([128, d], BF16, name="yb")
                        nc.vector.tensor_copy(yb, yf)
                        nc.sync.dma_start(out=y[r0:r0 + 128, :], in_=yb)
    return nc


# ---------------------------------------------------------------------------
# Host-side prep + gather
# ---------------------------------------------------------------------------

_PERM_HEAD = np.concatenate([np.arange(0, HD, 2), np.arange(1, HD, 2)])


def _bf16(a):
    """Fast float32 -> bfloat16 with round-to-nearest-even (bit twiddling —
    ~4x faster than ml_dtypes astype on large arrays)."""
    a = np.ascontiguousarray(a, dtype=np.float32)
    u = a.view(np.uint32)
    r = ((u >> 16) & 1) + np.uint32(0x7FFF)
    return ((u + r) >> 16).astype(np.uint16).view(NP_BF16).reshape(a.shape)


def _prep_in_maps(x, wq, wk, wv, wo, pos_cos, pos_sin, s=S):
    d = D
    lf = LF
    # permute q/k feature rows within each head: even pairs first, then odd
    wq_p = wq.reshape(N_HEADS, HD, d)[:, _PERM_HEAD, :].reshape(d, d)
    wk_p = wk.reshape(N_HEADS, HD, d)[:, _PERM_HEAD, :].reshape(d, d)
    wqT_full = _bf16(wq_p.T)
    wkT_full = _bf16(wk_p.T)
    wvT_full = _bf16(wv.T)
    woT_full = _bf16(wo.T)
    cs_half = pos_cos[0].T.astype(np.float32)  # [64, s]
    sn_half = pos_sin[0].T.astype(np.float32)
    csn = _bf16(np.concatenate([cs_half, cs_half, sn_half, -sn_half], axis=0))
    xT_all = _bf16(np.concatenate([x[b].T for b in range(x.shape[0])], axis=0))
    in_maps = []
    for c in range(N_CORES):
        g_in = np.ascontiguousarray(np.concatenate(
            [xT_all[c * XROWS:(c + 1) * XROWS],
             csn[c * CSROWS:(c + 1) * CSROWS]], axis=0))
        in_maps.append({
            "g_in": g_in,
            "wqT": np.ascontiguousarray(wqT_full[:, c * lf:(c + 1) * lf]),
            "wkT": np.ascontiguousarray(wkT_full[:, c * lf:(c + 1) * lf]),
            "wvT": np.ascontiguousarray(wvT_full[:, c * lf:(c + 1) * lf]),
            "woT": np.ascontiguousarray(woT_full[c * lf:(c + 1) * lf, :]),
        })
    return in_maps


_NC_CACHE = {}


def _get_nc(s=S):
    if s not in _NC_CACHE:
        _NC_CACHE[s] = build_nc(s)
    return _NC_CACHE[s]


def _np_rope(t, cos, sin):
    b, ss, hh, hd = t.shape
    tr = t.reshape(b, ss, hh, hd // 2, 2)
    te, to = tr[..., 0], tr[..., 1]
    c = cos[:, :, None, :]
    s = sin[:, :, None, :]
    return np.stack([te * c - to * s, te * s + to * c], axis=-1).reshape(b, ss, hh, hd)


def _score_sample_max(x, wq, wk, pos_cos, pos_sin):
    """Sampled estimate of max |score|; the device softmax skips the max
    subtraction, which is only safe when scores stay well under exp's fp32
    range."""
    ss = x[:, :: max(1, x.shape[1] // 32), :][:, :32]
    pos_idx = np.arange(x.shape[1])[:: max(1, x.shape[1] // 32)][:32]
    h = x.shape[2] // HD
    q = (ss @ wq.T).reshape(ss.shape[0], -1, h, HD)
    k = (ss @ wk.T).reshape(ss.shape[0], -1, h, HD)
    c = pos_cos[:, pos_idx]
    sn = pos_sin[:, pos_idx]
    q = _np_rope(q, c, sn)
    k = _np_rope(k, c, sn)
    sc = np.einsum('bqhd,bkhd->bhqk', q, k) / math.sqrt(HD)
    return float(np.abs(sc).max())


def _np_fallback(x, wq, wk, wv, wo, pos_cos, pos_sin):
    out = np.empty_like(x)
    h = x.shape[2] // HD
    for b in range(x.shape[0]):
        q = _np_rope((x[b:b + 1] @ wq.T).reshape(1, -1, h, HD), pos_cos, pos_sin)
        k = _np_rope((x[b:b + 1] @ wk.T).reshape(1, -1, h, HD), pos_cos, pos_sin)
        v = (x[b:b + 1] @ wv.T).reshape(1, -1, h, HD)
        sc = np.einsum('bqhd,bkhd->bhqk', q, k) / math.sqrt(HD)
        sc -= sc.max(axis=-1, keepdims=True)
        e = np.exp(sc, dtype=np.float32)
        p = e / e.sum(axis=-1, keepdims=True)
        out[b] = (np.einsum('bhqk,bkhd->bqhd', p, v).reshape(1, x.shape[1], -1)
                  @ wo.T)[0]
    return out


def kernel(x, wq, wk, wv, wo, pos_cos, pos_sin):
    x = np.asarray(x, dtype=np.float32)
    wq, wk, wv, wo = (np.asarray(a, dtype=np.float32) for a in (wq, wk, wv, wo))
    pos_cos = np.asarray(pos_cos, dtype=np.float32)
    pos_sin = np.asarray(pos_sin, dtype=np.float32)
    # the device softmax skips max subtraction (safe for scores ~ N(0,1));
    # if the inputs are scaled such that exp would overflow, fall back to a
    # correct (slower) host path rather than returning inf/NaN
    if 4.0 * _score_sample_max(x, wq, wk, pos_cos, pos_sin) > 80.0:
        return _np_fallback(x, wq, wk, wv, wo, pos_cos, pos_sin)
    s = x.shape[1]
    in_maps = _prep_in_maps(x, wq, wk, wv, wo, pos_cos, pos_sin, s=s)
    nc = _get_nc(s)
    res = run_bass_kernel_spmd(nc, in_maps, core_ids=list(range(N_CORES)))
    yb = np.concatenate([res.results[c]["y"] for c in range(N_CORES)], axis=0)
    return yb.astype(np.float32).reshape(B, s, D)


# revision 5
# speedup vs baseline: 1.6370x; 1.0142x over previous
"""Multi-head attention (RoPE, softmax, out-proj) on 8 Trainium2 NeuronCores.

The tunnel between host and the axon-attached devices runs at ~40-50 MB/s,
so the wall time of run_bass_kernel_spmd is dominated by bytes crossing it,
not by device compute. This kernel is organized so every byte crosses at
most once, quantized to 12 bits where that adds no error over bf16:

  - tensor-parallel over all 8 cores: core c owns heads {2c, 2c+1}
    (column-parallel wq/wk/wv, row-parallel wo), and processes BOTH batches
    for those heads. Weight slices are disjoint across cores.
  - x and the weights are shipped as fixed-point int12 (offset-2048) in two
    uint8 planes: a low-byte plane and a half-width packed-nibble plane
    (low nibbles = first half of the packing axis, high nibbles = second
    half, so device unpacking needs only contiguous-slice DVE ops). The
    quantization step (range/4096/sqrt(12)) is below bf16's own rounding
    noise, and the fixed scales fold into the exp-activation scale and the
    final output-convert multiply.
  - x planes + RoPE cos/sin rows are sharded row-wise 8 ways and
    AllGathered on device (on-chip collective, ~100us total) instead of
    being replicated over the tunnel.
  - each core's partial out-projection ([2S, D] f32, in raw quantized
    units) is ReduceScattered (add) across the 8 cores; each core rescales
    its [2S/8, D] shard to true units, converts to bf16, and returns only
    that. The host concatenates the shards.

Compute structure per core is the proven head-group pipeline (matmuls in
bf16 at full PE rate with fp32 PSUM accumulation; RoPE as a partition-block
half-swap with host-permuted q/k feature rows and [+sin;-sin] sign folding;
softmax unnormalized in exp with the denominator reduced by an fp32r
ones-matmul and applied as a reciprocal multiply).
"""
import math
import sys

import numpy as np

for _p in ('/opt/trn_rl_repo', '/root/.axon_site/_ro/trn_rl_repo'):
    if _p not in sys.path:
        sys.path.insert(0, _p)

import ml_dtypes
import orjson

import concourse.bass as bass
import concourse.mybir as mybir
from concourse.tile import TileContext
from concourse.bass_utils import run_bass_kernel_spmd

# The dispatch path builds a fresh jax.jit per call; without the persistent
# compilation cache each call re-lowers and re-compiles the wrapped NEFF
# executable (~0.4s). With it, steady-state calls hit the disk cache.
try:
    import jax as _jax
    _jax.config.update("jax_compilation_cache_dir", "/tmp/jax_comp_cache")
    _jax.config.update("jax_persistent_cache_min_entry_size_bytes", 0)
    _jax.config.update("jax_persistent_cache_min_compile_time_secs", 0.0)
except Exception:
    pass

F32 = mybir.dt.float32
R32 = mybir.dt.float32r
BF16 = mybir.dt.bfloat16
I16 = mybir.dt.int16
U8 = mybir.dt.uint8
NP_BF16 = ml_dtypes.bfloat16
ALU = mybir.AluOpType

B = 2
S = 2048
D = 2048
HD = 128
N_HEADS = D // HD   # 16
N_CORES = 8
HPC = N_HEADS // N_CORES   # heads per core (2)
LF = HPC * HD              # local features per core (256)
XROWS = B * D // N_CORES   # x-shard rows per core (512)
CSROWS = 2 * HD // N_CORES  # cos/sin shard rows per core (32)

# fixed int12 quantization scales (offset-2048 fixed point). x ~ N(0,1),
# w ~ N(0, 1/D): the ranges cover ~7 sigma (quantization step stays below
# bf16 rounding noise); host falls back to an exact path when inputs
# exceed them.
X_RANGE = 7.0
W_RANGE = 0.16
S_X = X_RANGE / 2047.0
S_W = W_RANGE / 2047.0


# ---------------------------------------------------------------------------
# Wait-splitting post-pass: this toolchain's walrus supports at most ONE sync
# wait command per instruction (none at all on fp32/fp32r Matmult, which
# lowers to an LDW+MM pair). Tile emits multi-wait instructions; hoist the
# excess onto NoOps on the same engine immediately before the instruction.
# ---------------------------------------------------------------------------

def _keep_count(ins):
    if ins.get('opcode') == 'Matmult':
        dt = None
        for arg in ins.get('ins', []):
            dt = arg.get('dtype') or dt
        if dt in ('float32', 'float32r'):
            return 0
        return 1
    return 1


def _split_waits_json(data: bytes) -> bytes:
    d = orjson.loads(data)
    ctr = 0
    for fn in d.get('functions', []):
        for bb in fn.get('blocks', []):
            out = []
            for ins in bb.get('instructions', []):
                si = ins.get('sync_info')
                waits = (si or {}).get('on_wait') or []
                keep = _keep_count(ins)
                if len(waits) > keep:
                    hoist = waits[:len(waits) - keep]
                    keep_w = waits[len(waits) - keep:]
                    for w in hoist:
                        ctr += 1
                        nop = {
                            'name': f"{ins['name']}-ws{ctr}",
                            'opcode': 'NoOp',
                            'engine': ins.get('engine'),
                            'ins': [],
                            'outs': [],
                            'sync_info': {'on_wait': [w], 'on_update': []},
                        }
                        if 'debug' in ins:
                            nop['debug'] = ins['debug']
                        out.append(nop)
                    si['on_wait'] = keep_w
                out.append(ins)
            bb['instructions'] = out
    return orjson.dumps(d)


def _install_waitsplit():
    if getattr(bass.Bass, '_waitsplit_installed', False):
        return
    orig = bass.Bass.to_json_bytes

    def patched(self, *a, **k):
        # memoized: the module is immutable once built, and the per-call jit
        # lowering re-serializes it otherwise (~0.1s each dispatch)
        if a or k:
            return _split_waits_json(orig(self, *a, **k))
        cached = getattr(self, '_json_bytes_cache', None)
        if cached is None:
            cached = _split_waits_json(orig(self))
            self._json_bytes_cache = cached
        return cached

    bass.Bass.to_json_bytes = patched
    bass.Bass._waitsplit_installed = True


_install_waitsplit()


# ---------------------------------------------------------------------------
# Device program (SPMD, identical on all cores; per-core data differs)
# ---------------------------------------------------------------------------

def build_nc(s=S):
    d = D
    lf = LF
    hpc = HPC
    kd_n = d // 128          # contraction chunks for projections (16)
    nw = 512 if s >= 512 else s  # free-dim width per matmul
    nsq = s // nw            # wide column chunks
    ns = s // 128            # 128-row chunks
    nj = d // 512
    jw = 512
    ry = B * s // N_CORES    # output rows per core after reduce-scatter
    scale = (S_X * S_W) ** 2 / math.sqrt(HD)  # exp() arg scale on raw scores
    c_y = S_X * S_W * S_W    # raw -> true units for the final output
    rg = [list(range(N_CORES))]
    sh = s // 2              # x nibble-plane width (packing along columns)

    nc = bass.Bass()
    x_lo = nc.dram_tensor("x_lo", [XROWS, s], U8, kind="ExternalInput")
    x_hi = nc.dram_tensor("x_hi", [XROWS, sh], U8, kind="ExternalInput")
    csn = nc.dram_tensor("csn", [CSROWS, s], BF16, kind="ExternalInput")
    # weights: int12 planes. wq/wk/wv pack along rows (D axis): hi-plane row
    # r holds nibbles of rows r (low) and r + d/2 (high). wo packs along its
    # lf rows: hi row r holds rows r (low) and r + lf/2 (high).
    wq_lo = nc.dram_tensor("wq_lo", [d, lf], U8, kind="ExternalInput")
    wq_hi = nc.dram_tensor("wq_hi", [d // 2, lf], U8, kind="ExternalInput")
    wk_lo = nc.dram_tensor("wk_lo", [d, lf], U8, kind="ExternalInput")
    wk_hi = nc.dram_tensor("wk_hi", [d // 2, lf], U8, kind="ExternalInput")
    wv_lo = nc.dram_tensor("wv_lo", [d, lf], U8, kind="ExternalInput")
    wv_hi = nc.dram_tensor("wv_hi", [d // 2, lf], U8, kind="ExternalInput")
    wo_lo = nc.dram_tensor("wo_lo", [lf, d], U8, kind="ExternalInput")
    wo_hi = nc.dram_tensor("wo_hi", [lf // 2, d], U8, kind="ExternalInput")
    y = nc.dram_tensor("y", [ry, d], BF16, kind="ExternalOutput")

    with TileContext(nc) as tc:
        with tc.tile_pool(name="dram", bufs=1, space="DRAM") as dpool:
            b_lo = dpool.tile([XROWS, s], U8, name="b_lo")
            b_hi = dpool.tile([XROWS, sh], U8, name="b_hi")
            b_csn = dpool.tile([CSROWS, s], BF16, name="b_csn")
            g_lo = dpool.tile([N_CORES * XROWS, s], U8,
                              addr_space="Shared", name="g_lo")
            g_hi = dpool.tile([N_CORES * XROWS, sh], U8,
                              addr_space="Shared", name="g_hi")
            g_csn = dpool.tile([N_CORES * CSROWS, s], BF16,
                               addr_space="Shared", name="g_csn")
            y_part = dpool.tile([B * s, d], F32, name="y_part")
            y_rs = dpool.tile([ry, d], F32, name="y_rs")

            nc.gpsimd.dma_start(out=b_lo[:], in_=x_lo[:])
            nc.gpsimd.dma_start(out=b_hi[:], in_=x_hi[:])
            nc.gpsimd.dma_start(out=b_csn[:], in_=csn[:])
            nc.gpsimd.collective_compute(
                "AllGather", ALU.bypass, replica_groups=rg,
                ins=[b_lo.opt()], outs=[g_lo.opt()])
            nc.gpsimd.collective_compute(
                "AllGather", ALU.bypass, replica_groups=rg,
                ins=[b_hi.opt()], outs=[g_hi.opt()])
            nc.gpsimd.collective_compute(
                "AllGather", ALU.bypass, replica_groups=rg,
                ins=[b_csn.opt()], outs=[g_csn.opt()])
            # after the gathers: g_lo/g_hi row i == xT_all row i (both
            # batches stacked), g_csn row i == csn_all row i.

            # Persistent SBUF residents: post-RoPE q/k and v for both batches
            # (4 virtual head-groups = 2 heads x 2 batches), and the fp32r
            # ones column used for the softmax denominator.
            with tc.tile_pool(name="persist", bufs=1) as per:
                qT_all = per.tile([128, B * hpc * s], BF16, name="qT_all")
                kT_all = per.tile([128, B * hpc * s], BF16, name="kT_all")
                v_all = per.tile([128, B * ns * lf], BF16, name="v_all")
                ones_f = per.tile([128, 128], F32, name="ones_f")
                nc.vector.memset(ones_f, 1.0)
                ones = per.tile([128, 128], R32, name="ones")
                nc.vector.tensor_copy(ones, ones_f)

                # ---------- Stage A0: unpack weights to SBUF (bf16 raw) ----------
                wsb_pool = tc.tile_pool(name="wsb", bufs=1)
                wpool = wsb_pool.__enter__()
                wq_sb = wpool.tile([128, kd_n * lf], BF16, name="wq_sb")
                wk_sb = wpool.tile([128, kd_n * lf], BF16, name="wk_sb")
                wv_sb = wpool.tile([128, kd_n * lf], BF16, name="wv_sb")
                wo_sb = wpool.tile([128, hpc * d], BF16, name="wo_sb")

                with tc.tile_pool(name="wscr", bufs=2) as wscr:
                    def unpack_w(dst, lo_t, hi_t, nchunks, width, engine):
                        # dst[:, k*width:(k+1)*width] <- rows [k*128, +128) of
                        # the logical [nchunks*128, width] int12 tensor whose
                        # planes are lo_t ([nchunks*128, width] u8) and hi_t
                        # ([nchunks*64... nchunks/2*128, width] u8, row-packed)
                        half = nchunks // 2
                        lo_sb = wscr.tile([128, nchunks * width], U8, name="w_lo_sb")
                        hi_sb = wscr.tile([128, half * width], U8, name="w_hi_sb")
                        for k in range(nchunks):
                            engine.dma_start(
                                out=lo_sb[:, k * width:(k + 1) * width],
                                in_=lo_t[k * 128:(k + 1) * 128, :])
                        for k in range(half):
                            engine.dma_start(
                                out=hi_sb[:, k * width:(k + 1) * width],
                                in_=hi_t[k * 128:(k + 1) * 128, :])
                        nibs = wscr.tile([128, nchunks * width], U8, name="w_nibs")
                        hw = half * width
                        nc.vector.tensor_scalar(nibs[:, 0:hw], hi_sb, 0x0F,
                                                None, ALU.bitwise_and)
                        nc.vector.tensor_scalar(nibs[:, hw:2 * hw], hi_sb, 4,
                                                None, ALU.logical_shift_right)
                        nib16 = wscr.tile([128, nchunks * width], I16, name="w_nib16")
                        nc.vector.tensor_scalar(nib16, nibs, 256, None, ALU.mult)
                        lo16 = wscr.tile([128, nchunks * width], I16, name="w_lo16")
                        nc.vector.tensor_copy(lo16, lo_sb)
                        nc.vector.tensor_add(lo16, lo16, nib16)
                        nc.vector.tensor_scalar(dst, lo16, 2048, None, ALU.subtract)

                    unpack_w(wq_sb, wq_lo, wq_hi, kd_n, lf, nc.sync)
                    unpack_w(wk_sb, wk_lo, wk_hi, kd_n, lf, nc.scalar)
                    unpack_w(wv_sb, wv_lo, wv_hi, kd_n, lf, nc.scalar)
                    unpack_w(wo_sb, wo_lo, wo_hi, hpc, d, nc.sync)

                # ---------- Stage A: q/k/v projections + RoPE ----------
                with tc.tile_pool(name="xa", bufs=2) as xpool, \
                     tc.tile_pool(name="xs", bufs=1) as xscr, \
                     tc.tile_pool(name="csp", bufs=1) as cspool, \
                     tc.tile_pool(name="rp", bufs=2) as rpool, \
                     tc.tile_pool(name="psA", bufs=4, space="PSUM") as pspool, \
                     tc.tile_pool(name="psAV", bufs=2, space="PSUM") as pvpool:
                    # cos/sin: gathered bf16 rows -> SBUF -> f32. sn rows are
                    # [+sin; -sin] (host-prepared) so the half-swap cross
                    # terms land with the right signs.
                    cs_bf = cspool.tile([128, s], BF16, name="cs_bf")
                    sn_bf = cspool.tile([128, s], BF16, name="sn_bf")
                    nc.sync.dma_start(out=cs_bf, in_=g_csn[0:128, :])
                    nc.sync.dma_start(out=sn_bf, in_=g_csn[128:256, :])
                    cs_sb = cspool.tile([128, s], F32, name="cs_sb")
                    sn_sb = cspool.tile([128, s], F32, name="sn_sb")
                    nc.vector.tensor_copy(cs_sb, cs_bf)
                    nc.vector.tensor_copy(sn_sb, sn_bf)

                    def load_unpack_x(b, sq):
                        # returns x_sb [128, kd_n*nw] bf16 raw-centered
                        c0 = sq * nw
                        # nibble side pieces of the value-column range
                        # [c0, c0+nw): (dst_off, src_col, width, side)
                        pieces = []
                        if c0 < sh:
                            w = min(c0 + nw, sh) - c0
                            pieces.append((0, c0, w, 'lo'))
                        if c0 + nw > sh:
                            st = max(c0, sh)
                            pieces.append((st - c0, st - sh, c0 + nw - st, 'hi'))
                        x_sb = xpool.tile([128, kd_n * nw], BF16, name="x_sb")
                        lo_sb = xscr.tile([128, kd_n * nw], U8, name="x_lo_sb")
                        hi_sb = xscr.tile([128, kd_n * nw], U8, name="x_hi_sb")
                        for kd in range(kd_n):
                            r = b * d + kd * 128
                            nc.sync.dma_start(
                                out=lo_sb[:, kd * nw:(kd + 1) * nw],
                                in_=g_lo[r:r + 128, c0:c0 + nw])
                            for off, sc, w, _side in pieces:
                                nc.sync.dma_start(
                                    out=hi_sb[:, kd * nw + off:kd * nw + off + w],
                                    in_=g_hi[r:r + 128, sc:sc + w])
                        # nibble extraction in place (each hi byte is used once)
                        if len(pieces) == 1:
                            op = ALU.bitwise_and if pieces[0][3] == 'lo' \
                                else ALU.logical_shift_right
                            arg = 0x0F if pieces[0][3] == 'lo' else 4
                            nc.vector.tensor_scalar(hi_sb, hi_sb, arg, None, op)
                        else:
                            for kd in range(kd_n):
                                for off, _sc, w, side in pieces:
                                    op = ALU.bitwise_and if side == 'lo' \
                                        else ALU.logical_shift_right
                                    arg = 0x0F if side == 'lo' else 4
                                    sl = slice(kd * nw + off, kd * nw + off + w)
                                    nc.vector.tensor_scalar(
                                        hi_sb[:, sl], hi_sb[:, sl], arg, None, op)
                        nib16 = xscr.tile([128, kd_n * nw], I16, name="x_nib16")
                        nc.vector.tensor_scalar(nib16, hi_sb, 256, None, ALU.mult)
                        lo16 = xscr.tile([128, kd_n * nw], I16, name="x_lo16")
                        nc.vector.tensor_copy(lo16, lo_sb)
                        nc.vector.tensor_add(lo16, lo16, nib16)
                        nc.vector.tensor_scalar(x_sb, lo16, 2048, None, ALU.subtract)
                        return x_sb

                    def emit_v(b, sq, x_tile):
                        for ss in range(nw // 128):
                            psv = pvpool.tile([128, lf], F32, name="psv")
                            for kd in range(kd_n):
                                nc.tensor.matmul(
                                    psv,
                                    x_tile[:, kd * nw + ss * 128:
                                           kd * nw + (ss + 1) * 128],
                                    wv_sb[:, kd * lf:(kd + 1) * lf],
                                    start=(kd == 0), stop=(kd == kd_n - 1))
                            sk = sq * (nw // 128) + ss
                            nc.vector.tensor_copy(
                                v_all[:, (b * ns + sk) * lf:(b * ns + sk + 1) * lf],
                                psv)

                    for bi in range(B):
                        for sq in range(nsq):
                            x_sb = load_unpack_x(bi, sq)
                            for wsb, dstT in ((wq_sb, qT_all), (wk_sb, kT_all)):
                                for h in range(hpc):
                                    g4 = bi * hpc + h
                                    ps = pspool.tile([128, nw], F32, name="ps_qk")
                                    for kd in range(kd_n):
                                        nc.tensor.matmul(
                                            ps,
                                            wsb[:, kd * lf + h * 128:
                                                kd * lf + (h + 1) * 128],
                                            x_sb[:, kd * nw:(kd + 1) * nw],
                                            start=(kd == 0), stop=(kd == kd_n - 1))
                                    tcc = rpool.tile([128, nw], F32, name="t_c")
                                    tss = rpool.tile([128, nw], F32, name="t_s")
                                    nc.vector.tensor_mul(
                                        tcc, ps, cs_sb[:, sq * nw:(sq + 1) * nw])
                                    nc.vector.tensor_mul(
                                        tss, ps, sn_sb[:, sq * nw:(sq + 1) * nw])
                                    tsw = rpool.tile([128, nw], F32, name="t_sw")
                                    nc.sync.dma_start(out=tsw[0:64, :],
                                                      in_=tss[64:128, :])
                                    nc.sync.dma_start(out=tsw[64:128, :],
                                                      in_=tss[0:64, :])
                                    nc.vector.tensor_add(
                                        dstT[:, g4 * s + sq * nw:
                                             g4 * s + sq * nw + nw], tcc, tsw)
                            emit_v(bi, sq, x_sb)

                # ---------- Stage B+C: attention, then partial out-proj ----------
                with tc.tile_pool(name="exp", bufs=2) as expool, \
                     tc.tile_pool(name="nrm", bufs=2) as npool, \
                     tc.tile_pool(name="atp", bufs=2) as atpool, \
                     tc.tile_pool(name="yop", bufs=3) as yopool, \
                     tc.tile_pool(name="psS", bufs=3, space="PSUM") as pssc, \
                     tc.tile_pool(name="psM", bufs=1, space="PSUM") as pssm, \
                     tc.tile_pool(name="psV", bufs=2, space="PSUM") as psov, \
                     tc.tile_pool(name="psC", bufs=2, space="PSUM") as psc:
                    nsub = nw // 128

                    def emit_c_part(bq, aT_tile, ssub):
                        # one query-row slice of the partial out-projection
                        bi, sq = bq
                        for jn in range(nj):
                            yps = psc.tile([128, jw], F32, name="yps")
                            for i in range(hpc):
                                nc.tensor.matmul(
                                    yps,
                                    aT_tile[:, i * nw + ssub * 128:
                                            i * nw + (ssub + 1) * 128],
                                    wo_sb[:, i * d + jn * jw:(i * d + (jn + 1) * jw)],
                                    start=(i == 0), stop=(i == hpc - 1))
                            yo = yopool.tile([128, jw], F32, name="yo")
                            nc.vector.tensor_copy(yo, yps)
                            r0 = bi * s + sq * nw + ssub * 128
                            nc.sync.dma_start(
                                out=y_part[r0:r0 + 128, jn * jw:(jn + 1) * jw],
                                in_=yo)

                    prev_c = None  # ((bi, sq), aT_tile) of the previous chunk
                    for bi in range(B):
                        for sq in range(nsq):
                            aT_sq = atpool.tile([128, hpc * nw], BF16, name="aT_sq")
                            for h in range(hpc):
                                g4 = bi * hpc + h
                                qT_sl = qT_all[:, g4 * s + sq * nw:
                                               g4 * s + (sq + 1) * nw]
                                ex_sb = expool.tile([128, ns * nw], BF16, name="ex_sb")
                                acc = npool.tile([128, nw], F32, name="acc")
                                pairs = []
                                for sk in range(ns):
                                    sps = pssc.tile([128, nw], F32, name="sps")
                                    nc.tensor.matmul(
                                        sps,
                                        kT_all[:, g4 * s + sk * 128:
                                               g4 * s + (sk + 1) * 128],
                                        qT_sl, start=True, stop=True)
                                    nc.scalar.activation(
                                        ex_sb[:, sk * nw:(sk + 1) * nw], sps,
                                        mybir.ActivationFunctionType.Exp,
                                        scale=scale)
                                    # pairwise level-0 exp sums on the
                                    # otherwise-idle GPSIMD engine
                                    if sk % 2 == 1:
                                        pr = npool.tile([128, nw], F32,
                                                        name=f"pr{sk // 2}")
                                        nc.gpsimd.tensor_add(
                                            pr, ex_sb[:, (sk - 1) * nw:sk * nw],
                                            ex_sb[:, sk * nw:(sk + 1) * nw])
                                        pairs.append(pr)
                                if ns == 1:
                                    nc.vector.tensor_copy(acc, ex_sb[:, 0:nw])
                                else:
                                    nc.vector.tensor_add(acc, pairs[0], pairs[1])
                                    for pr in pairs[2:]:
                                        nc.vector.tensor_add(acc, acc, pr)
                                ov = psov.tile([128, nw], F32, name="ov")
                                for sk in range(ns):
                                    nc.tensor.matmul(
                                        ov,
                                        v_all[:, (bi * ns + sk) * lf + h * 128:
                                              (bi * ns + sk) * lf + (h + 1) * 128],
                                        ex_sb[:, sk * nw:(sk + 1) * nw],
                                        start=(sk == 0), stop=(sk == ns - 1))
                                accr = npool.tile([128, nw], R32, name="accr")
                                nc.vector.tensor_copy(accr, acc)
                                # partition reduction + row broadcast of the
                                # softmax denominator
                                sm = pssm.tile([128, nw], F32, name="sm")
                                nc.tensor.matmul(sm, ones, accr, start=True,
                                                 stop=True)
                                rec = npool.tile([128, nw], F32, name="rec")
                                nc.vector.reciprocal(rec, sm)
                                nc.vector.tensor_mul(
                                    aT_sq[:, h * nw:(h + 1) * nw], ov, rec)
                                # interleave the PREVIOUS chunk's out-projection
                                # slices between heads
                                if prev_c is not None:
                                    pbq, pat = prev_c
                                    lo = h * nsub // hpc
                                    hi = (h + 1) * nsub // hpc
                                    for ssub in range(lo, hi):
                                        emit_c_part(pbq, pat, ssub)
                            prev_c = ((bi, sq), aT_sq)
                    pbq, pat = prev_c
                    for ssub in range(nsub):
                        emit_c_part(pbq, pat, ssub)

                wsb_pool.__exit__(None, None, None)

                # ---------- Stage D: reduce-scatter + rescale + bf16 ----------
                with tc.tile_pool(name="cvt", bufs=2) as cpool:
                    nc.gpsimd.collective_compute(
                        "ReduceScatter", ALU.add, replica_groups=rg,
                        ins=[y_part.opt()], outs=[y_rs.opt()])
                    for r0 in range(0, ry, 128):
                        yf = cpool.tile([128, d], F32, name="yf")
                        nc.sync.dma_start(out=yf, in_=y_rs[r0:r0 + 128, :])
                        yb = cpool.tile([128, d], BF16, name="yb")
                        nc.vector.tensor_scalar(yb, yf, c_y, None, ALU.mult)
                        nc.sync.dma_start(out=y[r0:r0 + 128, :], in_=yb)
    return nc


# ---------------------------------------------------------------------------
# Host-side prep + gather
# ---------------------------------------------------------------------------

_PERM_HEAD = np.concatenate([np.arange(0, HD, 2), np.arange(1, HD, 2)])


def _bf16(a):
    """Fast float32 -> bfloat16 with round-to-nearest-even (bit twiddling)."""
    a = np.ascontiguousarray(a, dtype=np.float32)
    u = a.view(np.uint32)
    r = ((u >> 16) & 1) + np.uint32(0x7FFF)
    return ((u + r) >> 16).astype(np.uint16).view(NP_BF16).reshape(a.shape)


def _pack12(a, scale, axis_len):
    """Quantize to offset-2048 int12 and split into (lo-byte plane,
    packed-nibble plane). Packing pairs row/col i (low nibble) with
    i + axis_len/2 (high nibble) along axis 0 of the 2-D array when
    axis_len == a.shape[0], else along axis 1."""
    u = (np.rint(a * (1.0 / scale)) + 2048.0)
    u = np.clip(u, 0.0, 4095.0).astype(np.uint16)
    lo = (u & 0xFF).astype(np.uint8)
    nib = (u >> 8).astype(np.uint8)
    h = axis_len // 2
    if axis_len == a.shape[0]:
        hi = nib[:h, :] | (nib[h:, :] << 4)
    else:
        hi = nib[:, :h] | (nib[:, h:] << 4)
    return lo, hi


def _prep_in_maps(x, wq, wk, wv, wo, pos_cos, pos_sin, s=S):
    d = D
    lf = LF
    # permute q/k feature rows within each head: even pairs first, then odd
    wq_p = wq.reshape(N_HEADS, HD, d)[:, _PERM_HEAD, :].reshape(d, d)
    wk_p = wk.reshape(N_HEADS, HD, d)[:, _PERM_HEAD, :].reshape(d, d)
    cs_half = pos_cos[0].T.astype(np.float32)  # [64, s]
    sn_half = pos_sin[0].T.astype(np.float32)
    csn = _bf16(np.concatenate([cs_half, cs_half, sn_half, -sn_half], axis=0))
    xT_all = np.concatenate([x[b].T for b in range(x.shape[0])], axis=0)
    xlo, xhi = _pack12(np.ascontiguousarray(xT_all), S_X, s)  # cols
    in_maps = []
    for c in range(N_CORES):
        # per-core weight slices: wq/wk/wv column-slices of W.T == row-slices
        # of W_p; transposed to [d, lf] then packed along rows
        m = {}
        for name, w_p in (("wq", wq_p), ("wk", wk_p), ("wv", wv)):
            wt = np.ascontiguousarray(w_p[c * lf:(c + 1) * lf, :].T)  # [d, lf]
            lo_p, hi_p = _pack12(wt, S_W, d)
            m[f"{name}_lo"], m[f"{name}_hi"] = lo_p, hi_p
        wot = np.ascontiguousarray(wo.T[c * lf:(c + 1) * lf, :])  # [lf, d]
        m["wo_lo"], m["wo_hi"] = _pack12(wot, S_W, lf)
        m["x_lo"] = np.ascontiguousarray(xlo[c * XROWS:(c + 1) * XROWS])
        m["x_hi"] = np.ascontiguousarray(xhi[c * XROWS:(c + 1) * XROWS])
        m["csn"] = np.ascontiguousarray(csn[c * CSROWS:(c + 1) * CSROWS])
        in_maps.append(m)
    return in_maps


_NC_CACHE = {}


def _get_nc(s=S):
    if s not in _NC_CACHE:
        _NC_CACHE[s] = build_nc(s)
    return _NC_CACHE[s]


def _np_rope(t, cos, sin):
    b, ss, hh, hd = t.shape
    tr = t.reshape(b, ss, hh, hd // 2, 2)
    te, to = tr[..., 0], tr[..., 1]
    c = cos[:, :, None, :]
    s = sin[:, :, None, :]
    return np.stack([te * c - to * s, te * s + to * c], axis=-1).reshape(b, ss, hh, hd)


def _score_sample_max(x, wq, wk, pos_cos, pos_sin):
    """Sampled estimate of max |score|; the device softmax skips the max
    subtraction, which is only safe when scores stay well under exp's fp32
    range."""
    ss = x[:, :: max(1, x.shape[1] // 32), :][:, :32]
    pos_idx = np.arange(x.shape[1])[:: max(1, x.shape[1] // 32)][:32]
    h = x.shape[2] // HD
    q = (ss @ wq.T).reshape(ss.shape[0], -1, h, HD)
    k = (ss @ wk.T).reshape(ss.shape[0], -1, h, HD)
    c = pos_cos[:, pos_idx]
    sn = pos_sin[:, pos_idx]
    q = _np_rope(q, c, sn)
    k = _np_rope(k, c, sn)
    sc = np.einsum('bqhd,bkhd->bhqk', q, k) / math.sqrt(HD)
    return float(np.abs(sc).max())


def _np_fallback(x, wq, wk, wv, wo, pos_cos, pos_sin):
    out = np.empty_like(x)
    h = x.shape[2] // HD
    for b in range(x.shape[0]):
        q = _np_rope((x[b:b + 1] @ wq.T).reshape(1, -1, h, HD), pos_cos, pos_sin)
        k = _np_rope((x[b:b + 1] @ wk.T).reshape(1, -1, h, HD), pos_cos, pos_sin)
        v = (x[b:b + 1] @ wv.T).reshape(1, -1, h, HD)
        sc = np.einsum('bqhd,bkhd->bhqk', q, k) / math.sqrt(HD)
        sc -= sc.max(axis=-1, keepdims=True)
        e = np.exp(sc, dtype=np.float32)
        p = e / e.sum(axis=-1, keepdims=True)
        out[b] = (np.einsum('bhqk,bkhd->bqhd', p, v).reshape(1, x.shape[1], -1)
                  @ wo.T)[0]
    return out


def kernel(x, wq, wk, wv, wo, pos_cos, pos_sin):
    x = np.asarray(x, dtype=np.float32)
    wq, wk, wv, wo = (np.asarray(a, dtype=np.float32) for a in (wq, wk, wv, wo))
    pos_cos = np.asarray(pos_cos, dtype=np.float32)
    pos_sin = np.asarray(pos_sin, dtype=np.float32)
    # guards: the int12 quantization ranges are fixed (baked into the NEFF),
    # and the device softmax skips max subtraction. Inputs outside either
    # envelope take a correct (slower) host path instead.
    if (np.abs(x).max() >= X_RANGE
            or max(np.abs(w).max() for w in (wq, wk, wv, wo)) >= W_RANGE
            or 4.0 * _score_sample_max(x, wq, wk, pos_cos, pos_sin) > 80.0):
        return _np_fallback(x, wq, wk, wv, wo, pos_cos, pos_sin)
    s = x.shape[1]
    in_maps = _prep_in_maps(x, wq, wk, wv, wo, pos_cos, pos_sin, s=s)
    nc = _get_nc(s)
    res = run_bass_kernel_spmd(nc, in_maps, core_ids=list(range(N_CORES)))
    yb = np.concatenate([res.results[c]["y"] for c in range(N_CORES)], axis=0)
    return yb.astype(np.float32).reshape(B, s, D)
